# revision 39
# baseline (speedup 1.0000x reference)
"""Bass/Trainium2 kernel for nn_EvoBinarizedLayer.

Reference computation (P=16 populations, B=512, I=O=2048, all values 0/1):
    out[p,b,o] = sum_i x[p,b,i]*w0[p,i,o] + (1-x[p,b,i])*w1[p,i,o]

Strategy (default builder: build_nc_v5, ~102us HW vs 128.6us baseline):
  - Shard population dim P across 8 cores (2 pops/core), embarrassingly parallel.
  - Algebraic rewrite: out = x@(w0-w1) + colsum(w1), halving the PE contraction
    vs the naive two-matmul form.
  - Host casts to fp8e4m3 and sends w0 and w1n = -w1 (+0.0 normalizes -0.0).
    Device computes wd = w0-w1 as a bitwise XOR of int32 views on the DVE:
    fp8(w0) XOR fp8(-w1) is bit-identical to fp8(w0-w1) for 0/1 weights
    ((1,1) gives 0x80 = -0, which accumulates as 0).  int32 XOR runs at
    4 fp8 bytes/lane/cycle, 4x the fp8 tensor_tensor rate that made the DVE
    the rate limiter in v4.
  - -bias = colsum(w1n) via an all-ones fp8 DoubleRow matmul (moving = w1n);
    evacuation is one DVE tensor_tensor subtract (psum - (-bias)) -> f16.
  - fp8 DoubleRow matmuls (K=256 per MM) hit the 157 TF/s fp8 peak (216ns
    per 512-col MM warm).
  - f16 output (integer sums <= 2048 are exact in f16) halves store traffic;
    host upcasts to f32 on gather.
  - A short warm-up matmul stream at t=0 holds the PE HAM clock gate at
    2.4 GHz before the first data-dependent matmuls issue; the final block's
    stores use the by-then-idle HWDGE rings to avoid the SWDGE end drain.
  - PSUM f32 accumulation of these integer products is exact, so the result
    is bit-exact vs the f32 reference (measured rel err 0.0).

Host-side work is layout only: slicing, transpose, dtype cast, and the final
gather. All arithmetic (notx, matmuls) happens on device.
"""

import os

import numpy as np
import ml_dtypes

from concourse import bacc, tile, mybir
from concourse.bass_utils import run_bass_kernel_spmd

P_TOT, B, I, O = 16, 512, 2048, 2048
N_CORES = 8
PPC = P_TOT // N_CORES  # pops per core = 2
PART = 128

FP8 = mybir.dt.float8e4
F32 = mybir.dt.float32
NP_FP8 = ml_dtypes.float8_e4m3


def build_nc(ppc=PPC, b=B, i_dim=I, o_dim=O, n_cores=N_CORES, use_dr=True):
    """Build + compile the per-core Bass program (SPMD: same program, 8 cores)."""
    kt = i_dim // PART          # k-subtiles per weight tensor (16)
    nb = o_dim // 512           # o-blocks (4)
    mb = b // PART              # b-subtiles (4)
    DR = mybir.MatmulPerfMode.DoubleRow if use_dr else None
    kstep = 2 if use_dr else 1

    nc = bacc.Bacc("TRN2", target_bir_lowering=False, debug=False,
                   num_devices=n_cores)

    xt_d = nc.dram_tensor("xt", [ppc, PART, kt, b], FP8, kind="ExternalInput")
    w0_d = nc.dram_tensor("w0", [ppc, nb, PART, kt, 512], FP8, kind="ExternalInput")
    w1_d = nc.dram_tensor("w1", [ppc, nb, PART, kt, 512], FP8, kind="ExternalInput")
    out_d = nc.dram_tensor("out", [ppc, b, o_dim], F32, kind="ExternalOutput")

    with tile.TileContext(nc) as tc:
        with (
            tc.tile_pool(name="warm", bufs=1) as warm,
            tc.tile_pool(name="xpool", bufs=2) as xpool,
            tc.tile_pool(name="wpool", bufs=8) as wpool,
            tc.tile_pool(name="opool", bufs=4) as opool,
            tc.tile_pool(name="pspool", bufs=4, space="PSUM") as pspool,
            tc.tile_pool(name="warmps", bufs=1, space="PSUM") as warmps,
        ):
            for pop in range(ppc):
                xt = xpool.tile([PART, kt, b], FP8, tag="xt")
                nxt = xpool.tile([PART, kt, b], FP8, tag="nxt")
                # x chunked on the scalar ring ahead of w1: the first matmul
                # needs only xt[:, 0:2, :], so a 256KB first chunk unblocks
                # the first LDWEIGHTS ~10us sooner than one 1MB transfer.
                xch = min(4, kt)
                for ch in range(0, kt, xch):
                    nc.scalar.dma_start(out=xt[:, ch:ch + xch, :],
                                        in_=xt_d.ap()[pop, :, ch:ch + xch, :])
                    # notx = 1 - x  ==  (x * -1) + 1, per chunk
                    nc.vector.tensor_scalar(
                        nxt[:, ch:ch + xch, :], xt[:, ch:ch + xch, :], -1.0, 1.0,
                        mybir.AluOpType.mult, mybir.AluOpType.add,
                    )
                for nbi in range(nb):
                    w0t = wpool.tile([PART, kt, 512], FP8, tag="w")
                    w1t = wpool.tile([PART, kt, 512], FP8, tag="w")
                    # w0 loads on the sync HWDGE ring, w1 on the scalar HWDGE
                    # ring (output stores go via gpsimd/SWDGE) so stores never
                    # block weight prefetch in a shared FIFO. Chunked k-wise so
                    # the first matmuls start before the whole block lands; the
                    # very first block uses finer chunks to cut the startup
                    # bubble before the first LDWEIGHTS.
                    wch = 2 if (pop == 0 and nbi == 0) else 4
                    for ch in range(0, kt, wch):
                        nc.sync.dma_start(
                            out=w0t[:, ch:ch + wch, :],
                            in_=w0_d.ap()[pop, nbi, :, ch:ch + wch, :])
                        nc.scalar.dma_start(
                            out=w1t[:, ch:ch + wch, :],
                            in_=w1_d.ap()[pop, nbi, :, ch:ch + wch, :])
                    for m in range(mb):
                        ps = pspool.tile([PART, 512], F32)
                        msl = slice(m * PART, (m + 1) * PART)
                        nk = kt // kstep
                        for kd in range(nk):
                            ksl = slice(kd * kstep, (kd + 1) * kstep)
                            nc.tensor.matmul(
                                ps[:], lhsT=xt[:, ksl, msl], rhs=w0t[:, ksl, :],
                                start=(kd == 0), stop=False, perf_mode=DR,
                            )
                        for kd in range(nk):
                            ksl = slice(kd * kstep, (kd + 1) * kstep)
                            nc.tensor.matmul(
                                ps[:], lhsT=nxt[:, ksl, msl], rhs=w1t[:, ksl, :],
                                start=False, stop=(kd == nk - 1), perf_mode=DR,
                            )
                        ot = opool.tile([PART, 512], F32)
                        nc.vector.tensor_copy(ot[:], ps[:])
                        nc.gpsimd.dma_start(
                            out=out_d.ap()[pop, msl, nbi * 512:(nbi + 1) * 512],
                            in_=ot[:],
                        )
    nc.compile()
    return nc


def build_nc_v3(ppc=PPC, b=B, i_dim=I, o_dim=O, n_cores=N_CORES):
    """v3: concat scheme (as v1) with stationary reuse.

    All weights for one population stay SBUF-resident (8MB fp8); the matmul
    loop is m -> half -> kd -> nb so one LDWEIGHTS serves 4 matmuls (one per
    o-block), cutting LDW traffic 4x and keeping the PE stream dense. PSUM
    holds 4 accumulating banks (one per o-block) per m-subtile.
    """
    kt = i_dim // PART
    nb = o_dim // 512
    mb = b // PART
    DR = mybir.MatmulPerfMode.DoubleRow
    nk = kt // 2

    nc = bacc.Bacc("TRN2", target_bir_lowering=False, debug=False,
                   num_devices=n_cores)

    xt_d = nc.dram_tensor("xt", [ppc, PART, kt, b], FP8, kind="ExternalInput")
    w0_d = nc.dram_tensor("w0", [ppc, nb, PART, kt, 512], FP8, kind="ExternalInput")
    w1_d = nc.dram_tensor("w1", [ppc, nb, PART, kt, 512], FP8, kind="ExternalInput")
    out_d = nc.dram_tensor("out", [ppc, b, o_dim], F32, kind="ExternalOutput")

    with tile.TileContext(nc) as tc:
        with (
            tc.tile_pool(name="xpool", bufs=2) as xpool,
            tc.tile_pool(name="wpool", bufs=2 * nb * 2) as wpool,
            tc.tile_pool(name="opool", bufs=6) as opool,
            tc.tile_pool(name="pspool", bufs=8, space="PSUM") as pspool,
        ):
            for pop in range(ppc):
                xt = xpool.tile([PART, kt, b], FP8, tag="xt")
                nxt = xpool.tile([PART, kt, b], FP8, tag="nxt")
                nc.gpsimd.dma_start(out=xt[:], in_=xt_d.ap()[pop])
                nc.vector.tensor_scalar(
                    nxt[:], xt[:], -1.0, 1.0,
                    mybir.AluOpType.mult, mybir.AluOpType.add,
                )
                # all weights for this pop, k-chunked so matmuls start early;
                # w0 on the sync HWDGE ring, w1 on the scalar HWDGE ring
                w0t = [wpool.tile([PART, kt, 512], FP8, tag="w",
                                  name=f"w0t_{pop}_{i}") for i in range(nb)]
                w1t = [wpool.tile([PART, kt, 512], FP8, tag="w",
                                  name=f"w1t_{pop}_{i}") for i in range(nb)]
                for ch in range(0, kt, 4):
                    for nbi in range(nb):
                        nc.sync.dma_start(
                            out=w0t[nbi][:, ch:ch + 4, :],
                            in_=w0_d.ap()[pop, nbi, :, ch:ch + 4, :])
                        nc.scalar.dma_start(
                            out=w1t[nbi][:, ch:ch + 4, :],
                            in_=w1_d.ap()[pop, nbi, :, ch:ch + 4, :])
                for m in range(mb):
                    msl = slice(m * PART, (m + 1) * PART)
                    pss = [pspool.tile([PART, 512], F32, tag="ps",
                                       name=f"ps_{pop}_{m}_{i}") for i in range(nb)]
                    for half, (xsrc, wt) in enumerate(((xt, w0t), (nxt, w1t))):
                        for kd in range(nk):
                            ksl = slice(2 * kd, 2 * kd + 2)
                            for nbi in range(nb):
                                nc.tensor.matmul(
                                    pss[nbi][:], lhsT=xsrc[:, ksl, msl],
                                    rhs=wt[nbi][:, ksl, :],
                                    start=(half == 0 and kd == 0),
                                    stop=(half == 1 and kd == nk - 1),
                                    perf_mode=DR,
                                )
                    for nbi in range(nb):
                        ot = opool.tile([PART, 512], F32)
                        nc.vector.tensor_copy(ot[:], pss[nbi][:])
                        nc.gpsimd.dma_start(
                            out=out_d.ap()[pop, msl, nbi * 512:(nbi + 1) * 512],
                            in_=ot[:],
                        )
    nc.compile()
    return nc


def build_nc_v4(ppc=PPC, b=B, i_dim=I, o_dim=O, n_cores=N_CORES):
    """v4: out = x@(w0-w1) + colsum(w1), wd built by DVE+gpsimd tensor_tensor.

    Halves the PE matmul stream vs the concat scheme (K=2048 instead of 4096).
    Per o-block: load w0/w1, bias = colsum(w1) via an all-ones DR matmul,
    wd = w0-w1 with the k-subtiles split between vector (11) and gpsimd (5)
    engines, main matmuls accumulate x@wd, and the DVE evacuation adds bias
    (tensor_tensor add against a bias tile copied from the bias PSUM bank).
    """
    kt = i_dim // PART
    nb = o_dim // 512
    mb = b // PART
    DR = mybir.MatmulPerfMode.DoubleRow
    nk = kt // 2
    # all subtract work on DVE: offloading 2 k-subtiles to gpsimd measured
    # 128.6us vs 128.0us all-DVE — the DVE's 23us of idle means it is not
    # strictly binding, and the gpsimd offload does not pay
    kdve = kt

    nc = bacc.Bacc("TRN2", target_bir_lowering=False, debug=False,
                   num_devices=n_cores)

    xt_d = nc.dram_tensor("xt", [ppc, PART, kt, b], FP8, kind="ExternalInput")
    w0_d = nc.dram_tensor("w0", [ppc, nb, PART, kt, 512], FP8, kind="ExternalInput")
    w1_d = nc.dram_tensor("w1", [ppc, nb, PART, kt, 512], FP8, kind="ExternalInput")
    out_d = nc.dram_tensor("out", [ppc, b, o_dim], F32, kind="ExternalOutput")

    with tile.TileContext(nc) as tc:
        with (
            tc.tile_pool(name="const", bufs=1) as const,
            tc.tile_pool(name="xpool", bufs=2) as xpool,
            tc.tile_pool(name="wsrc", bufs=6) as wsrc,
            tc.tile_pool(name="wdpool", bufs=4) as wdpool,
            tc.tile_pool(name="bpool", bufs=3) as bpool,
            tc.tile_pool(name="opool", bufs=4) as opool,
            tc.tile_pool(name="pspool", bufs=4, space="PSUM") as pspool,
            tc.tile_pool(name="psbias", bufs=2, space="PSUM") as psbias,
        ):
            ones = const.tile([PART, 2, PART], FP8)
            nc.vector.memset(ones[:], 1.0)
            xts = {}
            state = {}
            blocks = [(pop, nbi) for pop in range(ppc) for nbi in range(nb)]

            def prepare(pop, nbi):
                if nbi == 0:
                    xt = xpool.tile([PART, kt, b], FP8, tag="xt",
                                    name=f"xt_{pop}")
                    xch = min(4, kt)
                    for ch in range(0, kt, xch):
                        nc.scalar.dma_start(
                            out=xt[:, ch:ch + xch, :],
                            in_=xt_d.ap()[pop, :, ch:ch + xch, :])
                    xts[pop] = xt
                w0t = wsrc.tile([PART, kt, 512], FP8, tag="ws",
                                name=f"w0t_{pop}_{nbi}")
                w1t = wsrc.tile([PART, kt, 512], FP8, tag="ws",
                                name=f"w1t_{pop}_{nbi}")
                wch = 2 if (pop == 0 and nbi == 0) else 4
                for ch in range(0, kt, wch):
                    nc.sync.dma_start(
                        out=w1t[:, ch:ch + wch, :],
                        in_=w1_d.ap()[pop, nbi, :, ch:ch + wch, :])
                    nc.scalar.dma_start(
                        out=w0t[:, ch:ch + wch, :],
                        in_=w0_d.ap()[pop, nbi, :, ch:ch + wch, :])
                # bias = colsum(w1) (all rows of psb identical)
                psb = psbias.tile([PART, 512], F32, tag="psb")
                for kd in range(nk):
                    ksl = slice(2 * kd, 2 * kd + 2)
                    nc.tensor.matmul(
                        psb[:], lhsT=ones[:], rhs=w1t[:, ksl, :],
                        start=(kd == 0), stop=(kd == nk - 1), perf_mode=DR)
                bias_sb = bpool.tile([PART, 512], F32, tag="bias")
                nc.vector.tensor_copy(bias_sb[:], psb[:])
                # wd = w0 - w1 on DVE in fine k-chunks; emitted one block
                # AHEAD of the consuming matmuls (software pipeline) so these
                # sit before the previous block's evacuations in the DVE FIFO
                wd = wdpool.tile([PART, kt, 512], FP8, tag="wd")
                sch = max(1, kt // 8)
                for ch in range(0, kdve, sch):
                    nc.vector.tensor_tensor(
                        wd[:, ch:ch + sch, :], w0t[:, ch:ch + sch, :],
                        w1t[:, ch:ch + sch, :], mybir.AluOpType.subtract)
                if kdve < kt:
                    nc.gpsimd.tensor_tensor(
                        wd[:, kdve:, :], w0t[:, kdve:, :], w1t[:, kdve:, :],
                        mybir.AluOpType.subtract)
                state[(pop, nbi)] = (wd, bias_sb)

            def main(pop, nbi):
                wd, bias_sb = state.pop((pop, nbi))
                xt = xts[pop]
                for m in range(mb):
                    ps = pspool.tile([PART, 512], F32, tag="ps",
                                     name=f"ps_{pop}_{nbi}_{m}")
                    msl = slice(m * PART, (m + 1) * PART)
                    for kd in range(nk):
                        ksl = slice(2 * kd, 2 * kd + 2)
                        nc.tensor.matmul(
                            ps[:], lhsT=xt[:, ksl, msl], rhs=wd[:, ksl, :],
                            start=(kd == 0), stop=(kd == nk - 1), perf_mode=DR)
                    ot = opool.tile([PART, 512], F32, tag="ot",
                                    name=f"ot_{pop}_{nbi}_{m}")
                    nc.vector.tensor_tensor(
                        ot[:], ps[:], bias_sb[:], mybir.AluOpType.add)
                    nc.gpsimd.dma_start(
                        out=out_d.ap()[pop, msl, nbi * 512:(nbi + 1) * 512],
                        in_=ot[:])

            for i in range(len(blocks) + 1):
                if i < len(blocks):
                    prepare(*blocks[i])
                if i > 0:
                    main(*blocks[i - 1])
    nc.compile()
    return nc


def build_nc_v5(ppc=PPC, b=B, i_dim=I, o_dim=O, n_cores=N_CORES,
                warmup_mms=12, xor_chunk=4):
    """v5: out = x@wd + colsum(w1), wd built by int32 bitwise-XOR on DVE.

    Key trick: for 0/1 weights cast to fp8e4m3, fp8(w0) XOR fp8(-w1) is
    bit-identical to fp8(w0 - w1) in every case ((1,1) yields 0x80 = -0,
    which accumulates as 0).  The host sends w1n = -w1 (sign folded into
    the cast, +0.0 normalized), so the DVE computes wd with int32 bitwise
    XOR at 4 bytes/lane/cycle -- 4x the fp8 tensor_tensor rate that made
    v4's DVE the rate limiter (99us busy).

    Also: f16 output (exact for integer sums <= 2048, halves store
    traffic vs f32) and a PE warm-up matmul stream at t=0 so the HAM
    clock gate reaches 2.4 GHz before the real matmuls begin.
    """
    kt = i_dim // PART
    nb = o_dim // 512
    mb = b // PART
    DR = mybir.MatmulPerfMode.DoubleRow
    F16 = mybir.dt.float16
    I32 = mybir.dt.int32
    nk = kt // 2

    nc = bacc.Bacc("TRN2", target_bir_lowering=False, debug=False,
                   num_devices=n_cores)

    xt_d = nc.dram_tensor("xt", [ppc, PART, kt, b], FP8, kind="ExternalInput")
    w0_d = nc.dram_tensor("w0", [ppc, nb, PART, kt, 512], FP8, kind="ExternalInput")
    w1_d = nc.dram_tensor("w1", [ppc, nb, PART, kt, 512], FP8, kind="ExternalInput")
    out_d = nc.dram_tensor("out", [ppc, b, o_dim], F16, kind="ExternalOutput")

    with tile.TileContext(nc) as tc:
        with (
            tc.tile_pool(name="const", bufs=1) as const,
            tc.tile_pool(name="xpool", bufs=2) as xpool,
            tc.tile_pool(name="wsrc", bufs=4) as wsrc,
            tc.tile_pool(name="wdpool", bufs=4) as wdpool,
            tc.tile_pool(name="bpool", bufs=3) as bpool,
            tc.tile_pool(name="opool", bufs=6) as opool,
            tc.tile_pool(name="pspool", bufs=4, space="PSUM") as pspool,
            tc.tile_pool(name="psbias", bufs=2, space="PSUM") as psbias,
            tc.tile_pool(name="pswarm", bufs=1, space="PSUM") as pswarm,
        ):
            # --- PE warm-up: dummy matmuls from t~0 keep the PE busy while
            # the first weight DMAs land, so the HAM clock gate is at 8/8
            # (2.4 GHz) when the real stream begins.
            warm = const.tile([PART, 2, 512], FP8)
            nc.scalar.memzero(warm[:])
            psw = pswarm.tile([PART, 512], F32)
            for _ in range(warmup_mms):
                nc.tensor.matmul(psw[:], lhsT=warm[:, :, :PART], rhs=warm[:],
                                 start=True, stop=True, perf_mode=DR)

            ones = const.tile([PART, 2, PART], FP8)
            nc.vector.memset(ones[:], 1.0)
            xts = {}
            state = {}
            blocks = [(pop, nbi) for pop in range(ppc) for nbi in range(nb)]

            def prepare(pop, nbi):
                if nbi == 0:
                    xt = xpool.tile([PART, kt, b], FP8, tag="xt",
                                    name=f"xt_{pop}")
                    xch = min(4, kt)
                    for ch in range(0, kt, xch):
                        nc.scalar.dma_start(
                            out=xt[:, ch:ch + xch, :],
                            in_=xt_d.ap()[pop, :, ch:ch + xch, :])
                    xts[pop] = xt
                # w0 lands directly in the wd tile; w1n in its own tile.
                wd = wdpool.tile([PART, kt, 512], FP8, tag="wd",
                                 name=f"wd_{pop}_{nbi}")
                w1t = wsrc.tile([PART, kt, 512], FP8, tag="ws",
                                name=f"w1t_{pop}_{nbi}")
                wch = 2 if (pop == 0 and nbi == 0) else 4
                for ch in range(0, kt, wch):
                    nc.sync.dma_start(
                        out=w1t[:, ch:ch + wch, :],
                        in_=w1_d.ap()[pop, nbi, :, ch:ch + wch, :])
                    nc.scalar.dma_start(
                        out=wd[:, ch:ch + wch, :],
                        in_=w0_d.ap()[pop, nbi, :, ch:ch + wch, :])
                # -bias = colsum(w1n) via all-ones DR matmul (w1t holds -w1)
                psb = psbias.tile([PART, 512], F32, tag="psb")
                for kd in range(nk):
                    ksl = slice(2 * kd, 2 * kd + 2)
                    nc.tensor.matmul(
                        psb[:], lhsT=ones[:], rhs=w1t[:, ksl, :],
                        start=(kd == 0), stop=(kd == nk - 1), perf_mode=DR)
                # wd = w0 XOR w1n, int32 view: 4 fp8 bytes/lane/cycle.
                # Emitted BEFORE the bias copy so the DVE starts the XOR as
                # soon as the weights land (not serialized behind the bias
                # matmuls' PSUM result).
                for ch in range(0, kt, xor_chunk):
                    csl = slice(ch, ch + xor_chunk)
                    nc.vector.tensor_tensor(
                        wd[:, csl, :].bitcast(I32), wd[:, csl, :].bitcast(I32),
                        w1t[:, csl, :].bitcast(I32), mybir.AluOpType.bitwise_xor)
                bias_sb = bpool.tile([PART, 512], F32, tag="bias")
                nc.vector.tensor_copy(bias_sb[:], psb[:])
                state[(pop, nbi)] = (wd, bias_sb)

            def main(pop, nbi):
                wd, bias_sb = state.pop((pop, nbi))
                xt = xts[pop]
                for m in range(mb):
                    ps = pspool.tile([PART, 512], F32, tag="ps",
                                     name=f"ps_{pop}_{nbi}_{m}")
                    msl = slice(m * PART, (m + 1) * PART)
                    for kd in range(nk):
                        ksl = slice(2 * kd, 2 * kd + 2)
                        nc.tensor.matmul(
                            ps[:], lhsT=xt[:, ksl, msl], rhs=wd[:, ksl, :],
                            start=(kd == 0), stop=(kd == nk - 1), perf_mode=DR)
                    ot = opool.tile([PART, 512], F16, tag="ot",
                                    name=f"ot_{pop}_{nbi}_{m}")
                    # out = psum - (-bias)
                    nc.vector.tensor_tensor(
                        ot[:], ps[:], bias_sb[:], mybir.AluOpType.subtract)
                    # the final block's stores go on the (by now idle) HWDGE
                    # rings: ~0.6us completion vs SWDGE's ~1us + end drain
                    if pop == ppc - 1 and nbi == nb - 1:
                        eng = nc.sync if m % 2 == 0 else nc.scalar
                    else:
                        eng = nc.gpsimd
                    eng.dma_start(
                        out=out_d.ap()[pop, msl, nbi * 512:(nbi + 1) * 512],
                        in_=ot[:])

            for i in range(len(blocks) + 1):
                if i < len(blocks):
                    prepare(*blocks[i])
                if i > 0:
                    main(*blocks[i - 1])
    nc.compile()
    return nc


def build_nc_v9(ppc=PPC, b=B, i_dim=I, o_dim=O, n_cores=N_CORES,
                warmup_mms=8, xor_chunk=4):
    """v9: v5 with a 2-deep bias pipeline.

    PE order [bias0, bias1, main0, bias2, main1, ...]: during the DMA ramp
    the PE runs bias matmuls (which need only w1) instead of idling, and
    each block's w1 deadline moves a block earlier than its w0 deadline.
    Rings: sync = w1 (+ x tails), scalar = x head + w0 -- so w0 (the main
    matmul critical path, via XOR) never queues behind w1 bytes.
    Last block's stores go on the by-then-idle HWDGE rings.
    """
    kt = i_dim // PART
    nb = o_dim // 512
    mb = b // PART
    DR = mybir.MatmulPerfMode.DoubleRow
    F16 = mybir.dt.float16
    I32 = mybir.dt.int32
    nk = kt // 2

    nc = bacc.Bacc("TRN2", target_bir_lowering=False, debug=False,
                   num_devices=n_cores)

    xt_d = nc.dram_tensor("xt", [ppc, PART, kt, b], FP8, kind="ExternalInput")
    w0_d = nc.dram_tensor("w0", [ppc, nb, PART, kt, 512], FP8, kind="ExternalInput")
    w1_d = nc.dram_tensor("w1", [ppc, nb, PART, kt, 512], FP8, kind="ExternalInput")
    out_d = nc.dram_tensor("out", [ppc, b, o_dim], F16, kind="ExternalOutput")

    with tile.TileContext(nc) as tc:
        with (
            tc.tile_pool(name="const", bufs=1) as const,
            tc.tile_pool(name="xpool", bufs=2) as xpool,
            tc.tile_pool(name="wsrc", bufs=4) as wsrc,
            tc.tile_pool(name="wdpool", bufs=4) as wdpool,
            tc.tile_pool(name="bpool", bufs=3) as bpool,
            tc.tile_pool(name="opool", bufs=6) as opool,
            tc.tile_pool(name="pspool", bufs=4, space="PSUM") as pspool,
            tc.tile_pool(name="psbias", bufs=2, space="PSUM") as psbias,
            tc.tile_pool(name="pswarm", bufs=1, space="PSUM") as pswarm,
        ):
            warm = const.tile([PART, 2, 512], FP8)
            nc.scalar.memzero(warm[:])
            psw = pswarm.tile([PART, 512], F32)
            for _ in range(warmup_mms):
                nc.tensor.matmul(psw[:], lhsT=warm[:, :, :PART], rhs=warm[:],
                                 start=True, stop=True, perf_mode=DR)

            ones = const.tile([PART, 2, PART], FP8)
            nc.vector.memset(ones[:], 1.0)
            xts = {}
            state = {}
            blocks = [(pop, nbi) for pop in range(ppc) for nbi in range(nb)]
            nblocks = len(blocks)

            # x(0)'s first chunk leads the scalar ring (main(0) stationary)
            xt0 = xpool.tile([PART, kt, b], FP8, tag="xt", name="xt_0")
            xts[0] = xt0
            nc.scalar.dma_start(out=xt0[:, 0:4, :], in_=xt_d.ap()[0, :, 0:4, :])

            def prep_w1bias(i):
                pop, nbi = blocks[i]
                w1t = wsrc.tile([PART, kt, 512], FP8, tag="ws",
                                name=f"w1t_{pop}_{nbi}")
                wch = 2 if i == 0 else 4
                for ch in range(0, kt, wch):
                    nc.sync.dma_start(
                        out=w1t[:, ch:ch + wch, :],
                        in_=w1_d.ap()[pop, nbi, :, ch:ch + wch, :])
                if i == 1:   # x(0) tail on the w1 ring
                    nc.sync.dma_start(out=xt0[:, 4:kt, :],
                                      in_=xt_d.ap()[0, :, 4:kt, :])
                if i == 4 and ppc > 1:   # x(1) on the w1 ring
                    xt1 = xpool.tile([PART, kt, b], FP8, tag="xt", name="xt_1")
                    xts[1] = xt1
                    nc.sync.dma_start(out=xt1[:], in_=xt_d.ap()[1])
                # -bias = colsum(w1n) via all-ones DR matmul
                psb = psbias.tile([PART, 512], F32, tag="psb")
                for kd in range(nk):
                    ksl = slice(2 * kd, 2 * kd + 2)
                    nc.tensor.matmul(
                        psb[:], lhsT=ones[:], rhs=w1t[:, ksl, :],
                        start=(kd == 0), stop=(kd == nk - 1), perf_mode=DR)
                bias_sb = bpool.tile([PART, 512], F32, tag="bias")
                nc.vector.tensor_copy(bias_sb[:], psb[:])
                state[i] = (w1t, bias_sb)

            def prep_w0xor(i):
                pop, nbi = blocks[i]
                w1t, bias_sb = state[i]
                wd = wdpool.tile([PART, kt, 512], FP8, tag="wd",
                                 name=f"wd_{pop}_{nbi}")
                wch = 2 if i == 0 else 4
                for ch in range(0, kt, wch):
                    nc.scalar.dma_start(
                        out=wd[:, ch:ch + wch, :],
                        in_=w0_d.ap()[pop, nbi, :, ch:ch + wch, :])
                # wd = w0 XOR w1n (int32 view, 4 fp8 bytes/lane/cycle)
                for ch in range(0, kt, xor_chunk):
                    csl = slice(ch, ch + xor_chunk)
                    nc.vector.tensor_tensor(
                        wd[:, csl, :].bitcast(I32), wd[:, csl, :].bitcast(I32),
                        w1t[:, csl, :].bitcast(I32), mybir.AluOpType.bitwise_xor)
                state[i] = (wd, bias_sb)

            def main(i):
                pop, nbi = blocks[i]
                wd, bias_sb = state.pop(i)
                xt = xts[pop]
                for m in range(mb):
                    ps = pspool.tile([PART, 512], F32, tag="ps",
                                     name=f"ps_{pop}_{nbi}_{m}")
                    msl = slice(m * PART, (m + 1) * PART)
                    for kd in range(nk):
                        ksl = slice(2 * kd, 2 * kd + 2)
                        nc.tensor.matmul(
                            ps[:], lhsT=xt[:, ksl, msl], rhs=wd[:, ksl, :],
                            start=(kd == 0), stop=(kd == nk - 1), perf_mode=DR)
                    ot = opool.tile([PART, 512], F16, tag="ot",
                                    name=f"ot_{pop}_{nbi}_{m}")
                    # out = psum - (-bias)
                    nc.vector.tensor_tensor(
                        ot[:], ps[:], bias_sb[:], mybir.AluOpType.subtract)
                    if i == nblocks - 1:
                        eng = nc.sync if m % 2 == 0 else nc.scalar
                    else:
                        eng = nc.gpsimd
                    eng.dma_start(
                        out=out_d.ap()[pop, msl, nbi * 512:(nbi + 1) * 512],
                        in_=ot[:])

            prep_w1bias(0)
            prep_w0xor(0)
            prep_w1bias(1)
            for i in range(1, nblocks):
                main(i - 1)
                if i + 1 < nblocks:
                    prep_w1bias(i + 1)
                prep_w0xor(i)
            main(nblocks - 1)
    nc.compile()
    return nc


def build_nc_v7(ppc=PPC, b=B, i_dim=I, o_dim=O, n_cores=N_CORES,
                warmup_mms=12, xor_chunk=4):
    """v7: v5 structure (one-ahead prepare, lookahead-1 DMA) plus:
      - XOR emitted before the bias PSUM copy in the DVE queue, so it
        starts as soon as the weights land instead of serializing behind
        the bias matmuls' result;
      - pop 1's x loaded one block earlier (v5 stalled 3us on it);
      - the last two blocks' stores go on the by-then-idle HWDGE rings,
        avoiding the multi-us SWDGE drain after the final matmul.
    """
    kt = i_dim // PART
    nb = o_dim // 512
    mb = b // PART
    DR = mybir.MatmulPerfMode.DoubleRow
    F16 = mybir.dt.float16
    I32 = mybir.dt.int32
    nk = kt // 2
    nblocks = ppc * nb

    nc = bacc.Bacc("TRN2", target_bir_lowering=False, debug=False,
                   num_devices=n_cores)

    xt_d = nc.dram_tensor("xt", [ppc, PART, kt, b], FP8, kind="ExternalInput")
    w0_d = nc.dram_tensor("w0", [ppc, nb, PART, kt, 512], FP8, kind="ExternalInput")
    w1_d = nc.dram_tensor("w1", [ppc, nb, PART, kt, 512], FP8, kind="ExternalInput")
    out_d = nc.dram_tensor("out", [ppc, b, o_dim], F16, kind="ExternalOutput")

    with tile.TileContext(nc) as tc:
        with (
            tc.tile_pool(name="const", bufs=1) as const,
            tc.tile_pool(name="xpool", bufs=2) as xpool,
            tc.tile_pool(name="wsrc", bufs=4) as wsrc,
            tc.tile_pool(name="wdpool", bufs=4) as wdpool,
            tc.tile_pool(name="bpool", bufs=3) as bpool,
            tc.tile_pool(name="opool", bufs=6) as opool,
            tc.tile_pool(name="pspool", bufs=4, space="PSUM") as pspool,
            tc.tile_pool(name="psbias", bufs=2, space="PSUM") as psbias,
            tc.tile_pool(name="pswarm", bufs=1, space="PSUM") as pswarm,
        ):
            warm = const.tile([PART, 2, 512], FP8)
            nc.scalar.memzero(warm[:])
            psw = pswarm.tile([PART, 512], F32)
            for _ in range(warmup_mms):
                nc.tensor.matmul(psw[:], lhsT=warm[:, :, :PART], rhs=warm[:],
                                 start=True, stop=True, perf_mode=DR)

            ones = const.tile([PART, 2, PART], FP8)
            nc.vector.memset(ones[:], 1.0)
            xts = {}
            state = {}
            blocks = [(pop, nbi) for pop in range(ppc) for nbi in range(nb)]

            def load_x(pop):
                xt = xpool.tile([PART, kt, b], FP8, tag="xt",
                                name=f"xt_{pop}")
                xts[pop] = xt
                xch = min(4, kt)
                for ch in range(0, kt, xch):
                    nc.scalar.dma_start(
                        out=xt[:, ch:ch + xch, :],
                        in_=xt_d.ap()[pop, :, ch:ch + xch, :])

            def prepare(pop, nbi):
                if pop == 0 and nbi == 0:
                    load_x(0)
                wd = wdpool.tile([PART, kt, 512], FP8, tag="wd",
                                 name=f"wd_{pop}_{nbi}")
                w1t = wsrc.tile([PART, kt, 512], FP8, tag="ws",
                                name=f"w1t_{pop}_{nbi}")
                wch = 2 if (pop == 0 and nbi == 0) else 4
                for ch in range(0, kt, wch):
                    nc.sync.dma_start(
                        out=w1t[:, ch:ch + wch, :],
                        in_=w1_d.ap()[pop, nbi, :, ch:ch + wch, :])
                    nc.scalar.dma_start(
                        out=wd[:, ch:ch + wch, :],
                        in_=w0_d.ap()[pop, nbi, :, ch:ch + wch, :])
                if nbi == 3 and pop + 1 < ppc:
                    load_x(pop + 1)
                # -bias = colsum(w1n) via all-ones DR matmul
                psb = psbias.tile([PART, 512], F32, tag="psb")
                for kd in range(nk):
                    ksl = slice(2 * kd, 2 * kd + 2)
                    nc.tensor.matmul(
                        psb[:], lhsT=ones[:], rhs=w1t[:, ksl, :],
                        start=(kd == 0), stop=(kd == nk - 1), perf_mode=DR)
                bias_sb = bpool.tile([PART, 512], F32, tag="bias")
                nc.vector.tensor_copy(bias_sb[:], psb[:])
                state[(pop, nbi)] = (wd, w1t, bias_sb)

            def prep_xor(pop, nbi):
                # wd = w0 XOR w1n (int32 view, 4 fp8 bytes/lane/cycle).
                # Emitted AFTER main(i-1)'s evacuations in the DVE queue: a
                # DMA-gated op ahead of the evacs would back up PSUM and
                # stall the PE even when main(i-1)'s own data is ready.
                wd, w1t, bias_sb = state[(pop, nbi)]
                for ch in range(0, kt, xor_chunk):
                    csl = slice(ch, ch + xor_chunk)
                    nc.vector.tensor_tensor(
                        wd[:, csl, :].bitcast(I32), wd[:, csl, :].bitcast(I32),
                        w1t[:, csl, :].bitcast(I32), mybir.AluOpType.bitwise_xor)
                state[(pop, nbi)] = (wd, bias_sb)

            def main(pop, nbi):
                wd, bias_sb = state.pop((pop, nbi))
                xt = xts[pop]
                blk_i = pop * nb + nbi
                for m in range(mb):
                    ps = pspool.tile([PART, 512], F32, tag="ps",
                                     name=f"ps_{pop}_{nbi}_{m}")
                    msl = slice(m * PART, (m + 1) * PART)
                    for kd in range(nk):
                        ksl = slice(2 * kd, 2 * kd + 2)
                        nc.tensor.matmul(
                            ps[:], lhsT=xt[:, ksl, msl], rhs=wd[:, ksl, :],
                            start=(kd == 0), stop=(kd == nk - 1), perf_mode=DR)
                    ot = opool.tile([PART, 512], F16, tag="ot",
                                    name=f"ot_{pop}_{nbi}_{m}")
                    # out = psum - (-bias)
                    nc.vector.tensor_tensor(
                        ot[:], ps[:], bias_sb[:], mybir.AluOpType.subtract)
                    eng = nc.gpsimd
                    eng.dma_start(
                        out=out_d.ap()[pop, msl, nbi * 512:(nbi + 1) * 512],
                        in_=ot[:])

            for i in range(len(blocks) + 1):
                if i < len(blocks):
                    prepare(*blocks[i])
                if i > 0:
                    main(*blocks[i - 1])
                if i < len(blocks):
                    prep_xor(*blocks[i])
    nc.compile()
    return nc


def build_nc_v6(ppc=PPC, b=B, i_dim=I, o_dim=O, n_cores=N_CORES,
                warmup_mms=3, xor_chunk=4, lookahead=8, wch_steady=4,
                late_store_from=6):
    """v6: v5 with decoupled DMA lookahead.

    dma_block() emits only DMA traffic and runs `lookahead` blocks ahead
    of the PE/DVE stream, so HBM prefetch never falls behind the PE
    (v5's 15-40us stall cluster).  Block ordering on the scalar ring puts
    w0(0) before the bulk of x so the first XOR can start early; x's
    first chunk goes ahead of everything so main(0)'s stationary is
    ready.  Bias PSUM->SBUF copies move to the scalar engine (ACT is
    close to PSUM; DVE keeps only XOR + evacuation).
    """
    kt = i_dim // PART
    nb = o_dim // 512
    mb = b // PART
    DR = mybir.MatmulPerfMode.DoubleRow
    F16 = mybir.dt.float16
    I32 = mybir.dt.int32
    nk = kt // 2

    nc = bacc.Bacc("TRN2", target_bir_lowering=False, debug=False,
                   num_devices=n_cores)

    xt_d = nc.dram_tensor("xt", [ppc, PART, kt, b], FP8, kind="ExternalInput")
    w0_d = nc.dram_tensor("w0", [ppc, nb, PART, kt, 512], FP8, kind="ExternalInput")
    w1_d = nc.dram_tensor("w1", [ppc, nb, PART, kt, 512], FP8, kind="ExternalInput")
    out_d = nc.dram_tensor("out", [ppc, b, o_dim], F16, kind="ExternalOutput")

    with tile.TileContext(nc) as tc:
        with (
            tc.tile_pool(name="const", bufs=1) as const,
            tc.tile_pool(name="xpool", bufs=2) as xpool,
            tc.tile_pool(name="wsrc", bufs=min(lookahead + 2, 8)) as wsrc,
            tc.tile_pool(name="wdpool", bufs=min(lookahead + 2, 8)) as wdpool,
            tc.tile_pool(name="bpool", bufs=3) as bpool,
            tc.tile_pool(name="opool", bufs=6) as opool,
            tc.tile_pool(name="pspool", bufs=4, space="PSUM") as pspool,
            tc.tile_pool(name="psbias", bufs=2, space="PSUM") as psbias,
            tc.tile_pool(name="pswarm", bufs=1, space="PSUM") as pswarm,
        ):
            warm = const.tile([PART, 2, 512], FP8)
            nc.scalar.memzero(warm[:])
            psw = pswarm.tile([PART, 512], F32)
            for _ in range(warmup_mms):
                nc.tensor.matmul(psw[:], lhsT=warm[:, :, :PART], rhs=warm[:],
                                 start=True, stop=True, perf_mode=DR)

            ones = const.tile([PART, 2, PART], FP8)
            nc.vector.memset(ones[:], 1.0)
            xts = {}
            dstate = {}
            state = {}
            blocks = [(pop, nbi) for pop in range(ppc) for nbi in range(nb)]

            def load_x(pop):
                # split across both HWDGE rings to keep them balanced
                xt = xpool.tile([PART, kt, b], FP8, tag="xt",
                                name=f"xt_{pop}")
                xts[pop] = xt
                h = kt // 2
                nc.sync.dma_start(out=xt[:, 0:h, :],
                                  in_=xt_d.ap()[pop, :, 0:h, :])
                nc.scalar.dma_start(out=xt[:, h:kt, :],
                                    in_=xt_d.ap()[pop, :, h:kt, :])

            def dma_block(pop, nbi):
                first = (pop == 0 and nbi == 0)
                if first:
                    # first x chunk ahead of everything: main(0)'s stationary
                    xt = xpool.tile([PART, kt, b], FP8, tag="xt", name="xt_0")
                    xts[0] = xt
                    nc.scalar.dma_start(out=xt[:, 0:4, :],
                                        in_=xt_d.ap()[0, :, 0:4, :])
                wd = wdpool.tile([PART, kt, 512], FP8, tag="wd",
                                 name=f"wd_{pop}_{nbi}")
                w1t = wsrc.tile([PART, kt, 512], FP8, tag="ws",
                                name=f"w1t_{pop}_{nbi}")
                # chunk-interleave each tensor across BOTH rings so neither
                # ring ever carries more than half of any block's bytes --
                # the queues get equal SDMA service, so an imbalanced ring
                # directly delays its tensors (v6b regression)
                wch = 2 if first else wch_steady
                for j, ch in enumerate(range(0, kt, wch)):
                    e0, e1 = (nc.sync, nc.scalar) if j % 2 == 0 else                              (nc.scalar, nc.sync)
                    e0.dma_start(
                        out=w1t[:, ch:ch + wch, :],
                        in_=w1_d.ap()[pop, nbi, :, ch:ch + wch, :])
                    e1.dma_start(
                        out=wd[:, ch:ch + wch, :],
                        in_=w0_d.ap()[pop, nbi, :, ch:ch + wch, :])
                if first:
                    xt = xts[0]
                    nc.sync.dma_start(out=xt[:, 4:10, :],
                                      in_=xt_d.ap()[0, :, 4:10, :])
                    nc.scalar.dma_start(out=xt[:, 10:kt, :],
                                        in_=xt_d.ap()[0, :, 10:kt, :])
                elif nbi == 2 and pop + 1 < ppc:
                    # next pop's x after this block's weights: lands well
                    # before block (pop+1, 0) needs it
                    load_x(pop + 1)
                dstate[(pop, nbi)] = (wd, w1t)

            def pe_xor(pop, nbi):
                # wd = w0 XOR w1n, int32 view: 4 fp8 bytes/lane/cycle.
                # Emitted a full block ahead of the consuming matmuls, and
                # ahead of the previous block's evacuations in the DVE queue,
                # so it runs as soon as the weights land.
                wd, w1t = dstate[(pop, nbi)]
                for ch in range(0, kt, xor_chunk):
                    csl = slice(ch, ch + xor_chunk)
                    nc.vector.tensor_tensor(
                        wd[:, csl, :].bitcast(I32), wd[:, csl, :].bitcast(I32),
                        w1t[:, csl, :].bitcast(I32), mybir.AluOpType.bitwise_xor)

            def pe_bias(pop, nbi):
                # -bias = colsum(w1n) via all-ones DR matmul.  Emitted AFTER
                # main(i-1) so block i's w1 DMA deadline is a full block
                # later than the main matmuls that consume wd(i).
                wd, w1t = dstate.pop((pop, nbi))
                psb = psbias.tile([PART, 512], F32, tag="psb")
                for kd in range(nk):
                    ksl = slice(2 * kd, 2 * kd + 2)
                    nc.tensor.matmul(
                        psb[:], lhsT=ones[:], rhs=w1t[:, ksl, :],
                        start=(kd == 0), stop=(kd == nk - 1), perf_mode=DR)
                bias_sb = bpool.tile([PART, 512], F32, tag="bias")
                nc.vector.tensor_copy(bias_sb[:], psb[:])
                state[(pop, nbi)] = (wd, bias_sb)

            def main(pop, nbi):
                wd, bias_sb = state.pop((pop, nbi))
                xt = xts[pop]
                for m in range(mb):
                    ps = pspool.tile([PART, 512], F32, tag="ps",
                                     name=f"ps_{pop}_{nbi}_{m}")
                    msl = slice(m * PART, (m + 1) * PART)
                    for kd in range(nk):
                        ksl = slice(2 * kd, 2 * kd + 2)
                        nc.tensor.matmul(
                            ps[:], lhsT=xt[:, ksl, msl], rhs=wd[:, ksl, :],
                            start=(kd == 0), stop=(kd == nk - 1), perf_mode=DR)
                    ot = opool.tile([PART, 512], F16, tag="ot",
                                    name=f"ot_{pop}_{nbi}_{m}")
                    # out = psum - (-bias)
                    nc.vector.tensor_tensor(
                        ot[:], ps[:], bias_sb[:], mybir.AluOpType.subtract)
                    # late blocks store on the HWDGE rings (idle once the
                    # loads finish): avoids the multi-us SWDGE drain after
                    # the final matmul
                    blk_i = pop * nb + nbi
                    if blk_i >= late_store_from:
                        eng = nc.sync if m % 2 == 0 else nc.scalar
                    else:
                        eng = nc.gpsimd
                    eng.dma_start(
                        out=out_d.ap()[pop, msl, nbi * 512:(nbi + 1) * 512],
                        in_=ot[:])

            for i in range(min(lookahead, len(blocks))):
                dma_block(*blocks[i])
            # software pipeline, per iteration i:
            #   xor(i)     DVE -- before main(i-1)'s evacs in the DVE queue
            #   main(i-1)  PE stream + evac + store
            #   bias(i)    PE -- after main(i-1), relaxing w1(i)'s deadline
            for i in range(len(blocks) + 1):
                if i < len(blocks):
                    pe_xor(*blocks[i])
                    if i + lookahead < len(blocks):
                        dma_block(*blocks[i + lookahead])
                if i > 0:
                    main(*blocks[i - 1])
                if i < len(blocks):
                    pe_bias(*blocks[i])
    nc.compile()
    return nc


def build_nc_v10(ppc=PPC, b=B, i_dim=I, o_dim=O, n_cores=N_CORES,
                 warmup_mms=8, xor_chunk=2, xor_eng="vector",
                 psb_eng="vector", late_store_from=99, tree_eng="vector",
                 tree_split=0):
    """v10: full-prefetch + bias colsum off the PE + kd-outer main loop.

    Three structural changes vs v5/v7 (103-108us):
      1. ALL loads (x, w0, w1: 20.9MB/core) are issued up front in global
         deadline order, each tensor split half/half across the two HWDGE
         rings so both rings carry identical byte streams.  SBUF holds every
         weight tile (64KB/partition); the rings never idle and there is no
         per-block dependency stall on prefetch.
      2. bias = colsum(w1n) no longer streams all of w1 through the PE
         (8 DR MMs/block = 13.8us/core).  A single DVE tensor_tensor adds
         adjacent k-subtile pairs (w1 tile shaped [128, 8, 2, 512], exact in
         fp8: sums in [-2, 0]), then 4 short DR MMs reduce the 8 partials.
         PE bias cost drops 8.6us/core; DVE absorbs 2.9us/block.
      3. Main matmuls run kd-outer / m-inner over 4 concurrent PSUM banks,
         so each wd chunk is fully consumed as it lands: after the last
         weight byte of the kernel only ~4 matmuls remain (was ~25).
    Also: warm-up via vector.memset (scalar.memzero dragged in a 1.3us
    ACT_TABLE_LOAD before the first warm matmul), and the last block's
    stores ride the by-then-idle HWDGE rings.
    """
    kt = i_dim // PART          # 16
    nb = o_dim // 512           # 4
    mb = b // PART              # 4
    DR = mybir.MatmulPerfMode.DoubleRow
    F16 = mybir.dt.float16
    I32 = mybir.dt.int32
    nk = kt // 2                # 8 DR matmuls per (m, block)
    kh = kt // 2                # pair-groups per weight tile (8)
    nblocks = ppc * nb

    nc = bacc.Bacc("TRN2", target_bir_lowering=False, debug=False,
                   num_devices=n_cores)

    xt_d = nc.dram_tensor("xt", [ppc, PART, kt, b], FP8, kind="ExternalInput")
    # same bytes as [ppc, nb, 128, kt, 512]; the [kh, 2] split exposes
    # adjacent-pair adds as one multi-dim AP tensor_tensor
    w0_d = nc.dram_tensor("w0", [ppc, nb, PART, kh, 2, 512], FP8,
                          kind="ExternalInput")
    w1_d = nc.dram_tensor("w1", [ppc, nb, PART, kh, 2, 512], FP8,
                          kind="ExternalInput")
    out_d = nc.dram_tensor("out", [ppc, b, o_dim], F16, kind="ExternalOutput")

    with tile.TileContext(nc) as tc:
        with (
            tc.tile_pool(name="const", bufs=1) as const,
            tc.tile_pool(name="xpool", bufs=2) as xpool,
            tc.tile_pool(name="wsrc", bufs=nblocks) as wsrc,
            tc.tile_pool(name="wdpool", bufs=nblocks) as wdpool,
            tc.tile_pool(name="s8pool", bufs=3) as s8pool,
            tc.tile_pool(name="bpool", bufs=3) as bpool,
            tc.tile_pool(name="opool", bufs=12) as opool,
            tc.tile_pool(name="pspool", bufs=4, space="PSUM") as pspool,
            tc.tile_pool(name="psbias", bufs=2, space="PSUM") as psbias,
            tc.tile_pool(name="pswarm", bufs=1, space="PSUM") as pswarm,
        ):
            # PE warm-up from t~0 (vector memset: no ACT table load)
            warm = const.tile([PART, 2, 512], FP8)
            nc.vector.memset(warm[:], 0.0)
            ones = const.tile([PART, 2, PART], FP8)
            nc.vector.memset(ones[:], 1.0)
            psw = pswarm.tile([PART, 512], F32)
            for _ in range(warmup_mms):
                nc.tensor.matmul(psw[:], lhsT=warm[:, :, :PART], rhs=warm[:],
                                 start=True, stop=True, perf_mode=DR)

            blocks = [(pop, nbi) for pop in range(ppc) for nbi in range(nb)]

            # ---- all loads up front, deadline order, half per HWDGE ring.
            # Block 0 lands in fine chunks so the PE can chase it; the rest
            # are single 512KB halves (best SDMA efficiency).
            xts = [xpool.tile([PART, kt, b], FP8, tag="xt", name=f"xt_{p}")
                   for p in range(ppc)]
            w1t = [wsrc.tile([PART, kh, 2, 512], FP8, tag="ws",
                             name=f"w1t_{i}") for i in range(nblocks)]
            wd4 = [wdpool.tile([PART, kh, 2, 512], FP8, tag="wd",
                               name=f"wd_{i}") for i in range(nblocks)]

            def load_w(i, chunks):
                pop, nbi = blocks[i]
                c0 = 0
                for ch in chunks:   # ch = number of kh pair-groups
                    h = ch // 2
                    nc.sync.dma_start(
                        out=w1t[i][:, c0:c0 + h, :, :],
                        in_=w1_d.ap()[pop, nbi, :, c0:c0 + h, :, :])
                    nc.scalar.dma_start(
                        out=w1t[i][:, c0 + h:c0 + ch, :, :],
                        in_=w1_d.ap()[pop, nbi, :, c0 + h:c0 + ch, :, :])
                    nc.sync.dma_start(
                        out=wd4[i][:, c0:c0 + h, :, :],
                        in_=w0_d.ap()[pop, nbi, :, c0:c0 + h, :, :])
                    nc.scalar.dma_start(
                        out=wd4[i][:, c0 + h:c0 + ch, :, :],
                        in_=w0_d.ap()[pop, nbi, :, c0 + h:c0 + ch, :, :])
                    c0 += ch

            def load_x(p, k0, k1):
                h = (k0 + k1) // 2
                nc.sync.dma_start(out=xts[p][:, k0:h, :],
                                  in_=xt_d.ap()[p, :, k0:h, :])
                nc.scalar.dma_start(out=xts[p][:, h:k1, :],
                                    in_=xt_d.ap()[p, :, h:k1, :])

            load_w(0, [2, 2, 4])    # w1/w0 block 0: {2,2,4} pair-groups
            load_x(0, 0, 4)         # x0 head: subtiles 0-3
            load_x(0, 4, 10)
            load_x(0, 10, kt)
            load_w(1, [4, 4])
            for i in range(2, nblocks):
                if i == 4 and ppc > 1:
                    load_x(1, 0, kt)
                load_w(i, [kh])

            # ---- per-block compute chain
            state = {}

            def prep_bias_xor(i):
                pop, nbi = blocks[i]
                # s8[j] = w1n[2j] + w1n[2j+1]  (fp8-exact: values in [-2,0])
                s8 = s8pool.tile([PART, kh, 512], FP8, tag="s8",
                                 name=f"s8_{i}")
                teng = {"vector": nc.vector, "gpsimd": nc.gpsimd}[tree_eng]
                if i == 0:
                    for c0, c1 in ((0, 2), (2, 4), (4, kh)):
                        teng.tensor_tensor(
                            s8[:, c0:c1, :], w1t[i][:, c0:c1, 0, :],
                            w1t[i][:, c0:c1, 1, :], mybir.AluOpType.add)
                elif tree_split:
                    h = tree_split
                    nc.gpsimd.tensor_tensor(
                        s8[:, :h, :], w1t[i][:, :h, 0, :],
                        w1t[i][:, :h, 1, :], mybir.AluOpType.add)
                    nc.vector.tensor_tensor(
                        s8[:, h:, :], w1t[i][:, h:, 0, :],
                        w1t[i][:, h:, 1, :], mybir.AluOpType.add)
                else:
                    teng.tensor_tensor(
                        s8[:], w1t[i][:, :, 0, :], w1t[i][:, :, 1, :],
                        mybir.AluOpType.add)
                # -bias = colsum(s8) via 4 short DR matmuls
                psb = psbias.tile([PART, 512], F32, tag="psb",
                                  name=f"psb_{i}")
                for j in range(kh // 2):
                    nc.tensor.matmul(
                        psb[:], lhsT=ones[:], rhs=s8[:, 2 * j:2 * j + 2, :],
                        start=(j == 0), stop=(j == kh // 2 - 1), perf_mode=DR)
                bias_sb = bpool.tile([PART, 512], F32, tag="bias",
                                     name=f"bias_{i}")
                if psb_eng == "scalar":
                    nc.scalar.copy(bias_sb[:], psb[:])
                else:
                    nc.vector.tensor_copy(bias_sb[:], psb[:])
                # wd = w0 XOR w1n (int32 view): fp8(w0) ^ fp8(-w1) is
                # bit-identical to fp8(w0-w1) for 0/1 weights
                xeng = {"vector": nc.vector, "gpsimd": nc.gpsimd}[xor_eng]
                for c in range(0, kh, xor_chunk):
                    csl = slice(c, c + xor_chunk)
                    xeng.tensor_tensor(
                        wd4[i][:, csl, :, :].bitcast(I32),
                        wd4[i][:, csl, :, :].bitcast(I32),
                        w1t[i][:, csl, :, :].bitcast(I32),
                        mybir.AluOpType.bitwise_xor)
                state[i] = bias_sb

            def main(i):
                pop, nbi = blocks[i]
                bias_sb = state.pop(i)
                xt = xts[pop]
                wd = wd4[i]
                pss = [pspool.tile([PART, 512], F32, tag="ps",
                                   name=f"ps_{i}_{m}") for m in range(mb)]
                osl = slice(nbi * 512, (nbi + 1) * 512)
                for kd in range(nk):
                    for m in range(mb):
                        msl = slice(m * PART, (m + 1) * PART)
                        nc.tensor.matmul(
                            pss[m][:], lhsT=xt[:, 2 * kd:2 * kd + 2, msl],
                            rhs=wd[:, kd, :, :],
                            start=(kd == 0), stop=(kd == nk - 1),
                            perf_mode=DR)
                        if kd == nk - 1:
                            # evac chases the stops; bank m is free again
                            # ~3 matmuls later for the next block
                            ot = opool.tile([PART, 512], F16, tag="ot",
                                            name=f"ot_{i}_{m}")
                            nc.vector.tensor_tensor(
                                ot[:], pss[m][:], bias_sb[:],
                                mybir.AluOpType.subtract)
                            if i >= late_store_from or i == nblocks - 1:
                                eng = nc.sync if m % 2 == 0 else nc.scalar
                            else:
                                eng = nc.gpsimd
                            eng.dma_start(out=out_d.ap()[pop, msl, osl],
                                          in_=ot[:])

            prep_bias_xor(0)
            for i in range(nblocks):
                main(i)
                if i + 1 < nblocks:
                    prep_bias_xor(i + 1)
    nc.compile()
    return nc


def build_nc_v12(ppc=PPC, b=B, i_dim=I, o_dim=O, n_cores=N_CORES,
                 warmup_mms=4, bit_dtype="int16", late_store_from=5,
                 tree_pairs=8):
    """v12: v10 structure with the DVE/ACT/PE work rebalanced.

    (The scaled {0,+-128} all-bitwise tree was tried and is mathematically
    dead: this fp8 is IEEE e4m3, max 240, and the byte trick inherently
    lands on exp=1111 = inf.  Bytes were verified identical to the fp8-add
    path on device, so int16 bitwise TT + custom-imm STT do work on DVE.)

    vs v10:
      - bias pair-tree: one fp8 TT add per block on DVE (3.57us measured);
        bias finals are 4 short DR MMs.
      - XOR runs on int16 views (2-byte dtype qualifies for the DVE 2x
        packed perf mode; int32 gets none).
      - psb -> bias_sb copy moves to ACT.
      - Block 0's bias streams w1 through the PE directly (raw v5-style
        MMs double as clock warm-up while block 0 prefetches).
      - All weight loads on the sync ring in strict deadline order (a
        single HWDGE queue sustains ~425 GB/s); x loads ride scalar.
      - Last block runs m-outer so only one evac+store trails the last MM.
    """
    kt = i_dim // PART
    nb = o_dim // 512
    mb = b // PART
    DR = mybir.MatmulPerfMode.DoubleRow
    F16 = mybir.dt.float16
    BIT = {"int16": mybir.dt.int16, "int32": mybir.dt.int32}[bit_dtype]
    SHIFT_OR = mybir.AluOpType.logical_shift_right
    nk = kt // 2
    kh = kt // 2
    nblocks = ppc * nb

    nc = bacc.Bacc("TRN2", target_bir_lowering=False, debug=False,
                   num_devices=n_cores)

    # x is m-major: [ppc, mb, PART, kt, 128] so each 0.25MB m-slice can land
    # at its own deadline (block 0 runs m-outer and starts on slice 0)
    xt_d = nc.dram_tensor("xt", [ppc, mb, PART, kt, PART], FP8,
                          kind="ExternalInput")
    w0_d = nc.dram_tensor("w0", [ppc, nb, PART, kh, 2, 512], FP8,
                          kind="ExternalInput")
    w1_d = nc.dram_tensor("w1", [ppc, nb, PART, kh, 2, 512], FP8,
                          kind="ExternalInput")
    out_d = nc.dram_tensor("out", [ppc, b, o_dim], F16, kind="ExternalOutput")

    with tile.TileContext(nc) as tc:
        with (
            tc.tile_pool(name="const", bufs=1) as const,
            tc.tile_pool(name="xpool", bufs=2 * mb) as xpool,
            tc.tile_pool(name="wsrc", bufs=nblocks) as wsrc,
            tc.tile_pool(name="wdpool", bufs=nblocks) as wdpool,
            tc.tile_pool(name="s8pool", bufs=3) as s8pool,
            tc.tile_pool(name="bpool", bufs=3) as bpool,
            tc.tile_pool(name="opool", bufs=12) as opool,
            tc.tile_pool(name="pspool", bufs=4, space="PSUM") as pspool,
            tc.tile_pool(name="psbias", bufs=2, space="PSUM") as psbias,
            tc.tile_pool(name="pswarm", bufs=1, space="PSUM") as pswarm,
        ):
            warm = const.tile([PART, 2, 512], FP8)
            nc.vector.memset(warm[:], 0.0)
            ones = const.tile([PART, 2, PART], FP8)
            nc.vector.memset(ones[:], 1.0)
            psw = pswarm.tile([PART, 512], F32)
            for _ in range(warmup_mms):
                nc.tensor.matmul(psw[:], lhsT=warm[:, :, :PART], rhs=warm[:],
                                 start=True, stop=True, perf_mode=DR)

            blocks = [(pop, nbi) for pop in range(ppc) for nbi in range(nb)]
            # per-m x tiles: 3-d APs for lhsT (int-indexing a 4-d tile for
            # lhsT mis-slices), and each 0.25MB m-slice lands independently
            xts = [[xpool.tile([PART, kt, PART], FP8, tag="xt",
                               name=f"xt_{p}_{m}") for m in range(mb)]
                   for p in range(ppc)]
            w1t = [wsrc.tile([PART, kh, 2, 512], FP8, tag="ws",
                             name=f"w1t_{i}") for i in range(nblocks)]
            wd4 = [wdpool.tile([PART, kh, 2, 512], FP8, tag="wd",
                               name=f"wd_{i}") for i in range(nblocks)]

            # ---- loads: weights on sync (strict deadline order), x on scalar
            def load_w(i, chunks):
                pop, nbi = blocks[i]
                c0 = 0
                for ch in chunks:
                    nc.sync.dma_start(
                        out=w1t[i][:, c0:c0 + ch, :, :],
                        in_=w1_d.ap()[pop, nbi, :, c0:c0 + ch, :, :])
                    c0 += ch
                c0 = 0
                for ch in chunks:
                    nc.sync.dma_start(
                        out=wd4[i][:, c0:c0 + ch, :, :],
                        in_=w0_d.ap()[pop, nbi, :, c0:c0 + ch, :, :])
                    c0 += ch

            # deadline order on the single weight ring: x0 m-slice 0 first
            # (block 0 is m-outer), block-0 weights in fine chunks, then the
            # rest of x0, then blocks in order with x1 before pop 1's blocks
            nc.sync.dma_start(out=xts[0][0][:], in_=xt_d.ap()[0, 0])
            load_w(0, [1, 1, 2, 4])
            for m in range(1, mb):
                nc.sync.dma_start(out=xts[0][m][:], in_=xt_d.ap()[0, m])
            for i in range(1, nblocks):
                if i == nb and ppc > 1:
                    for m in range(mb):
                        nc.sync.dma_start(out=xts[1][m][:],
                                          in_=xt_d.ap()[1, m])
                load_w(i, [kh])

            state = {}

            def prep(i):
                pop, nbi = blocks[i]
                psb = psbias.tile([PART, 512], F32, tag="psb",
                                  name=f"psb_{i}")
                if i == 0:
                    # raw bias: stream w1 through the PE (doubles as warm-up)
                    for j in range(kh):
                        nc.tensor.matmul(
                            psb[:], lhsT=ones[:], rhs=w1t[i][:, j, :, :],
                            start=(j == 0), stop=(j == kh - 1), perf_mode=DR)
                else:
                    # split bias between DVE pair-tree (tp pairs) and raw PE
                    # streaming (the rest): minimizes max(PE, DVE) per block
                    tp = tree_pairs
                    nmm = (tp // 2) + (kh - tp)
                    mmi = 0
                    if tp:
                        s8 = s8pool.tile([PART, tp, 512], FP8, tag="s8",
                                         name=f"s8_{i}")
                        nc.vector.tensor_tensor(
                            s8[:], w1t[i][:, 0:tp, 0, :], w1t[i][:, 0:tp, 1, :],
                            mybir.AluOpType.add)
                        for j in range(tp // 2):
                            nc.tensor.matmul(
                                psb[:], lhsT=ones[:],
                                rhs=s8[:, 2 * j:2 * j + 2, :],
                                start=(mmi == 0), stop=(mmi == nmm - 1),
                                perf_mode=DR)
                            mmi += 1
                    for j in range(tp, kh):
                        nc.tensor.matmul(
                            psb[:], lhsT=ones[:], rhs=w1t[i][:, j, :, :],
                            start=(mmi == 0), stop=(mmi == nmm - 1),
                            perf_mode=DR)
                        mmi += 1
                bias_sb = bpool.tile([PART, 512], F32, tag="bias",
                                     name=f"bias_{i}")
                nc.scalar.copy(bias_sb[:], psb[:])
                # wd = w0 XOR w1n (single op: saves per-op issue overhead)
                if i == 0:
                    for c in range(0, kh, 2):
                        nc.vector.tensor_tensor(
                            wd4[i][:, c:c + 2, :, :].bitcast(BIT),
                            wd4[i][:, c:c + 2, :, :].bitcast(BIT),
                            w1t[i][:, c:c + 2, :, :].bitcast(BIT),
                            mybir.AluOpType.bitwise_xor)
                else:
                    nc.vector.tensor_tensor(
                        wd4[i][:].bitcast(BIT), wd4[i][:].bitcast(BIT),
                        w1t[i][:].bitcast(BIT), mybir.AluOpType.bitwise_xor)
                state[i] = bias_sb

            def main(i):
                pop, nbi = blocks[i]
                bias_sb = state.pop(i)
                xt = xts[pop]
                wd = wd4[i]
                pss = [pspool.tile([PART, 512], F32, tag="ps",
                                   name=f"ps_{i}_{m}") for m in range(mb)]
                osl = slice(nbi * 512, (nbi + 1) * 512)

                def evac(m):
                    ot = opool.tile([PART, 512], F16, tag="ot",
                                    name=f"ot_{i}_{m}")
                    # out = ps - (-bias)
                    nc.vector.tensor_tensor(
                        ot[:], pss[m][:], bias_sb[:],
                        mybir.AluOpType.subtract)
                    msl = slice(m * PART, (m + 1) * PART)
                    if i >= late_store_from:
                        eng = nc.sync if m % 2 == 0 else nc.scalar
                    else:
                        eng = nc.gpsimd
                    eng.dma_start(out=out_d.ap()[pop, msl, osl], in_=ot[:])

                if i == nblocks - 1 or i == 0:
                    # m-outer: block 0 starts on x m-slice 0 before the rest
                    # of x lands; last block leaves only one evac+store
                    # trailing the final matmul
                    for m in range(mb):
                        for kd in range(nk):
                            nc.tensor.matmul(
                                pss[m][:], lhsT=xt[m][:, 2 * kd:2 * kd + 2, :],
                                rhs=wd[:, kd, :, :], start=(kd == 0),
                                stop=(kd == nk - 1), perf_mode=DR)
                        evac(m)
                else:
                    for kd in range(nk):
                        for m in range(mb):
                            nc.tensor.matmul(
                                pss[m][:], lhsT=xt[m][:, 2 * kd:2 * kd + 2, :],
                                rhs=wd[:, kd, :, :], start=(kd == 0),
                                stop=(kd == nk - 1), perf_mode=DR)
                            if kd == nk - 1:
                                evac(m)

            prep(0)
            for i in range(nblocks):
                main(i)
                if i + 1 < nblocks:
                    prep(i + 1)
    nc.compile()
    return nc


def build_nc_v2(ppc=PPC, b=B, i_dim=I, o_dim=O, n_cores=N_CORES):
    """v2: algebraic rewrite out = x@(w0-w1) + colsum(w1).

    The w1 input tensor holds -w1 (sign applied during the host fp8 cast;
    walrus rejects cce_op=subtract but accepts add):
    - wd = w0 + (-w1) computed by the gpsimd DMA inline ALU (accum_op=add)
      while loading w0 — zero compute-engine cost.
    - colsum(-w1) = -bias via an all-ones stationary matmul against the tile
      while it still holds -w1, once per o-block.
    - main pass: psum = x @ wd, half the PE work of v1; evacuated as
      psum - (-bias) with a DVE tensor_tensor subtract.
    All values stay exact: x in {0,1}, wd in {-1,0,1} (fp8 exact), bias and
    accumulation in f32 (integers < 2^24).
    """
    kt = i_dim // PART
    nb = o_dim // 512
    mb = b // PART
    DR = mybir.MatmulPerfMode.DoubleRow
    nk = kt // 2

    nc = bacc.Bacc("TRN2", target_bir_lowering=False, debug=False,
                   num_devices=n_cores)

    xt_d = nc.dram_tensor("xt", [ppc, PART, kt, b], FP8, kind="ExternalInput")
    w0_d = nc.dram_tensor("w0", [ppc, nb, PART, kt, 512], FP8, kind="ExternalInput")
    w1_d = nc.dram_tensor("w1", [ppc, nb, PART, kt, 512], FP8, kind="ExternalInput")
    out_d = nc.dram_tensor("out", [ppc, b, o_dim], F32, kind="ExternalOutput")

    with tile.TileContext(nc) as tc:
        with (
            tc.tile_pool(name="const", bufs=1) as const,
            tc.tile_pool(name="xpool", bufs=2) as xpool,
            tc.tile_pool(name="wpool", bufs=4) as wpool,
            tc.tile_pool(name="bpool", bufs=2) as bpool,
            tc.tile_pool(name="opool", bufs=4) as opool,
            tc.tile_pool(name="pspool", bufs=4, space="PSUM") as pspool,
            tc.tile_pool(name="psbias", bufs=2, space="PSUM") as psbias,
        ):
            ones = const.tile([PART, 2, PART], FP8)
            nc.vector.memset(ones[:], 1.0)
            for pop in range(ppc):
                xt = xpool.tile([PART, kt, b], FP8, tag="xt")
                nc.scalar.dma_start(out=xt[:], in_=xt_d.ap()[pop])
                for nbi in range(nb):
                    # 544-wide rows (512 data + 32 pad): keeps every SBUF write
                    # run at 512B so the accum DMA's RMW ucode accepts it (runs
                    # >512B crash the exec unit), and stops the AP optimizer
                    # from merging rows into one big run.
                    wdp = wpool.tile([PART, kt, 544], FP8, tag="w")
                    wd = wdp[:, :, :512]
                    # 1) load -w1 (sync HWDGE ring)
                    wch = min(8, kt)
                    for ch in range(0, kt, wch):
                        nc.sync.dma_start(
                            out=wd[:, ch:ch + wch, :],
                            in_=w1_d.ap()[pop, nbi, :, ch:ch + wch, :])
                    # 2) -bias = colsum(-w1) while the tile still holds -w1
                    psb = psbias.tile([PART, 512], F32)
                    for kd in range(nk):
                        ksl = slice(2 * kd, 2 * kd + 2)
                        nc.tensor.matmul(
                            psb[:], lhsT=ones[:], rhs=wd[:, ksl, :],
                            start=(kd == 0), stop=(kd == nk - 1), perf_mode=DR)
                    bias_sb = bpool.tile([PART, 512], F32, tag="bias")
                    nc.vector.tensor_copy(bias_sb[:], psb[:])
                    # 3) wd = w0 + (-w1) via DMA inline ALU (op(in,out) = in+out)
                    nc.gpsimd.dma_start(out=wd[:], in_=w0_d.ap()[pop, nbi],
                                        accum_op=mybir.AluOpType.add)
                    # 4) main pass: psum = x @ wd, evac with bias add
                    for m in range(mb):
                        ps = pspool.tile([PART, 512], F32)
                        msl = slice(m * PART, (m + 1) * PART)
                        for kd in range(nk):
                            ksl = slice(2 * kd, 2 * kd + 2)
                            nc.tensor.matmul(
                                ps[:], lhsT=xt[:, ksl, msl], rhs=wd[:, ksl, :],
                                start=(kd == 0), stop=(kd == nk - 1), perf_mode=DR)
                        ot = opool.tile([PART, 512], F32)
                        # out = psum - (-bias)
                        nc.vector.tensor_tensor(
                            ot[:], ps[:], bias_sb[:], mybir.AluOpType.subtract)
                        nc.scalar.dma_start(
                            out=out_d.ap()[pop, msl, nbi * 512:(nbi + 1) * 512],
                            in_=ot[:])
    nc.compile()
    return nc


def prep_core_inputs(x, w, core, ppc=PPC, negate_w1=False, wscale=1.0,
                     x_mmajor=False):
    """Layout-only host prep for one core: slice pops, transpose x, tile, cast.
    With negate_w1, the fp8 cast of w1 carries a sign flip (v2 sends -w1 so the
    device can form w0-w1 with the DMA ALU's accum add).  wscale selects the
    fp8 code pair used for the 0/1 booleans (v12 uses {0,+-128} so the device
    bias tree is bitwise); the device folds the 2^-7 back in during evac."""
    p0 = core * ppc
    b, i_dim = x.shape[1], x.shape[2]
    o_dim = w.shape[4]
    kt = i_dim // PART
    nb = o_dim // 512
    xs = x[p0:p0 + ppc]                       # [ppc, B, I]
    if x_mmajor:
        # [ppc, mb, 128, kt, 128]; xm[p,m,kp,kti,j] = x[p, m*128+j, kti*128+kp]
        mb = b // PART
        xt = np.ascontiguousarray(
            xs.reshape(ppc, mb, PART, kt, PART).transpose(0, 1, 4, 3, 2)
        ).astype(NP_FP8)
    else:
        # xT partition-tiled: [ppc, 128, kt, B];  xt[p, kp, kti, b] = x[p, b, kti*128+kp]
        xt = np.ascontiguousarray(
            xs.reshape(ppc, b, kt, PART).transpose(0, 3, 2, 1)
        ).astype(NP_FP8)
    ws = w[:, p0:p0 + ppc, 0]                 # [2, ppc, I, O]
    # [2, ppc, nb, 128, kt, 512]; wt[j,p,nbi,kp,kti,no] = w[j,p,kti*128+kp, nbi*512+no]
    wt = np.ascontiguousarray(
        ws.reshape(2, ppc, kt, PART, nb, 512).transpose(0, 1, 4, 3, 2, 5)
    )
    w0 = (wt[0] * wscale).astype(NP_FP8) if wscale != 1.0 else wt[0].astype(NP_FP8)
    # +0.0 normalizes -0.0 so the fp8 pattern is 0x00, not 0x80 -- the
    # XOR identity requires w1n in {+0.0, -scale} exactly.
    w1 = ((wt[1] * -wscale) + 0.0).astype(NP_FP8) if negate_w1 else wt[1].astype(NP_FP8)
    return {"xt": xt, "w0": w0, "w1": w1}


_NC_CACHE = {}

# which builder kernel() uses: 1 = concat (x@w0 + notx@w1), 2 = DMA-subtract trick
K_VERSION = int(os.environ.get("EVO_KERNEL_VERSION", "10"))
NEGATE_VERSIONS = (2, 5, 6, 7, 9, 10, 11, 12, 13)
RESHAPE_VERSIONS = (10, 11, 12, 13)
SCALE128_VERSIONS = ()
XMMAJOR_VERSIONS = (12, 13)


def _get_nc():
    if "nc" not in _NC_CACHE:
        builder = {1: build_nc, 2: build_nc_v2, 3: build_nc_v3,
                   4: build_nc_v4, 5: build_nc_v5, 6: build_nc_v6,
                   7: build_nc_v7, 9: build_nc_v9, 10: build_nc_v10,
                   11: lambda: build_nc_v10(tree_eng="gpsimd",
                                            psb_eng="scalar",
                                            late_store_from=5),
                   12: build_nc_v12,
                   13: lambda: build_nc_v12(tree_pairs=4)}[K_VERSION]
        _NC_CACHE["nc"] = builder()
    return _NC_CACHE["nc"]


def _reshape_for_v10(m):
    # v10 declares w0/w1 as [ppc, nb, 128, kh, 2, 512] (same bytes)
    for k in ("w0", "w1"):
        s = m[k].shape
        m[k] = m[k].reshape(s[0], s[1], s[2], s[3] // 2, 2, s[4])
    return m


def kernel(x, w):
    x = np.asarray(x)
    w = np.asarray(w)
    nc = _get_nc()
    wscale = 128.0 if K_VERSION in SCALE128_VERSIONS else 1.0
    in_maps = [prep_core_inputs(x, w, c,
                                negate_w1=(K_VERSION in NEGATE_VERSIONS),
                                wscale=wscale,
                                x_mmajor=(K_VERSION in XMMAJOR_VERSIONS))
               for c in range(N_CORES)]
    if K_VERSION in RESHAPE_VERSIONS:
        in_maps = [_reshape_for_v10(m) for m in in_maps]
    res = run_bass_kernel_spmd(nc, in_maps, list(range(N_CORES)))
    out = np.concatenate([res.results[c]["out"] for c in range(N_CORES)], axis=0)
    return np.ascontiguousarray(out.astype(np.float32))



# revision 45
# speedup vs baseline: 1.0404x; 1.0404x over previous
"""Bass/Trainium2 kernel for nn_EvoBinarizedLayer.

Reference computation (P=16 populations, B=512, I=O=2048, all values 0/1):
    out[p,b,o] = sum_i x[p,b,i]*w0[p,i,o] + (1-x[p,b,i])*w1[p,i,o]

Strategy (default builder: build_nc_v5, ~102us HW vs 128.6us baseline):
  - Shard population dim P across 8 cores (2 pops/core), embarrassingly parallel.
  - Algebraic rewrite: out = x@(w0-w1) + colsum(w1), halving the PE contraction
    vs the naive two-matmul form.
  - Host casts to fp8e4m3 and sends w0 and w1n = -w1 (+0.0 normalizes -0.0).
    Device computes wd = w0-w1 as a bitwise XOR of int32 views on the DVE:
    fp8(w0) XOR fp8(-w1) is bit-identical to fp8(w0-w1) for 0/1 weights
    ((1,1) gives 0x80 = -0, which accumulates as 0).  int32 XOR runs at
    4 fp8 bytes/lane/cycle, 4x the fp8 tensor_tensor rate that made the DVE
    the rate limiter in v4.
  - -bias = colsum(w1n) via an all-ones fp8 DoubleRow matmul (moving = w1n);
    evacuation is one DVE tensor_tensor subtract (psum - (-bias)) -> f16.
  - fp8 DoubleRow matmuls (K=256 per MM) hit the 157 TF/s fp8 peak (216ns
    per 512-col MM warm).
  - f16 output (integer sums <= 2048 are exact in f16) halves store traffic;
    host upcasts to f32 on gather.
  - A short warm-up matmul stream at t=0 holds the PE HAM clock gate at
    2.4 GHz before the first data-dependent matmuls issue; the final block's
    stores use the by-then-idle HWDGE rings to avoid the SWDGE end drain.
  - PSUM f32 accumulation of these integer products is exact, so the result
    is bit-exact vs the f32 reference (measured rel err 0.0).

Host-side work is layout only: slicing, transpose, dtype cast, and the final
gather. All arithmetic (notx, matmuls) happens on device.
"""

import os

import numpy as np
import ml_dtypes

from concourse import bacc, tile, mybir
from concourse.bass_utils import run_bass_kernel_spmd

P_TOT, B, I, O = 16, 512, 2048, 2048
N_CORES = 8
PPC = P_TOT // N_CORES  # pops per core = 2
PART = 128

FP8 = mybir.dt.float8e4
F32 = mybir.dt.float32
NP_FP8 = ml_dtypes.float8_e4m3


def build_nc(ppc=PPC, b=B, i_dim=I, o_dim=O, n_cores=N_CORES, use_dr=True):
    """Build + compile the per-core Bass program (SPMD: same program, 8 cores)."""
    kt = i_dim // PART          # k-subtiles per weight tensor (16)
    nb = o_dim // 512           # o-blocks (4)
    mb = b // PART              # b-subtiles (4)
    DR = mybir.MatmulPerfMode.DoubleRow if use_dr else None
    kstep = 2 if use_dr else 1

    nc = bacc.Bacc("TRN2", target_bir_lowering=False, debug=False,
                   num_devices=n_cores)

    xt_d = nc.dram_tensor("xt", [ppc, PART, kt, b], FP8, kind="ExternalInput")
    w0_d = nc.dram_tensor("w0", [ppc, nb, PART, kt, 512], FP8, kind="ExternalInput")
    w1_d = nc.dram_tensor("w1", [ppc, nb, PART, kt, 512], FP8, kind="ExternalInput")
    out_d = nc.dram_tensor("out", [ppc, b, o_dim], F32, kind="ExternalOutput")

    with tile.TileContext(nc) as tc:
        with (
            tc.tile_pool(name="warm", bufs=1) as warm,
            tc.tile_pool(name="xpool", bufs=2) as xpool,
            tc.tile_pool(name="wpool", bufs=8) as wpool,
            tc.tile_pool(name="opool", bufs=4) as opool,
            tc.tile_pool(name="pspool", bufs=4, space="PSUM") as pspool,
            tc.tile_pool(name="warmps", bufs=1, space="PSUM") as warmps,
        ):
            for pop in range(ppc):
                xt = xpool.tile([PART, kt, b], FP8, tag="xt")
                nxt = xpool.tile([PART, kt, b], FP8, tag="nxt")
                # x chunked on the scalar ring ahead of w1: the first matmul
                # needs only xt[:, 0:2, :], so a 256KB first chunk unblocks
                # the first LDWEIGHTS ~10us sooner than one 1MB transfer.
                xch = min(4, kt)
                for ch in range(0, kt, xch):
                    nc.scalar.dma_start(out=xt[:, ch:ch + xch, :],
                                        in_=xt_d.ap()[pop, :, ch:ch + xch, :])
                    # notx = 1 - x  ==  (x * -1) + 1, per chunk
                    nc.vector.tensor_scalar(
                        nxt[:, ch:ch + xch, :], xt[:, ch:ch + xch, :], -1.0, 1.0,
                        mybir.AluOpType.mult, mybir.AluOpType.add,
                    )
                for nbi in range(nb):
                    w0t = wpool.tile([PART, kt, 512], FP8, tag="w")
                    w1t = wpool.tile([PART, kt, 512], FP8, tag="w")
                    # w0 loads on the sync HWDGE ring, w1 on the scalar HWDGE
                    # ring (output stores go via gpsimd/SWDGE) so stores never
                    # block weight prefetch in a shared FIFO. Chunked k-wise so
                    # the first matmuls start before the whole block lands; the
                    # very first block uses finer chunks to cut the startup
                    # bubble before the first LDWEIGHTS.
                    wch = 2 if (pop == 0 and nbi == 0) else 4
                    for ch in range(0, kt, wch):
                        nc.sync.dma_start(
                            out=w0t[:, ch:ch + wch, :],
                            in_=w0_d.ap()[pop, nbi, :, ch:ch + wch, :])
                        nc.scalar.dma_start(
                            out=w1t[:, ch:ch + wch, :],
                            in_=w1_d.ap()[pop, nbi, :, ch:ch + wch, :])
                    for m in range(mb):
                        ps = pspool.tile([PART, 512], F32)
                        msl = slice(m * PART, (m + 1) * PART)
                        nk = kt // kstep
                        for kd in range(nk):
                            ksl = slice(kd * kstep, (kd + 1) * kstep)
                            nc.tensor.matmul(
                                ps[:], lhsT=xt[:, ksl, msl], rhs=w0t[:, ksl, :],
                                start=(kd == 0), stop=False, perf_mode=DR,
                            )
                        for kd in range(nk):
                            ksl = slice(kd * kstep, (kd + 1) * kstep)
                            nc.tensor.matmul(
                                ps[:], lhsT=nxt[:, ksl, msl], rhs=w1t[:, ksl, :],
                                start=False, stop=(kd == nk - 1), perf_mode=DR,
                            )
                        ot = opool.tile([PART, 512], F32)
                        nc.vector.tensor_copy(ot[:], ps[:])
                        nc.gpsimd.dma_start(
                            out=out_d.ap()[pop, msl, nbi * 512:(nbi + 1) * 512],
                            in_=ot[:],
                        )
    nc.compile()
    return nc


def build_nc_v3(ppc=PPC, b=B, i_dim=I, o_dim=O, n_cores=N_CORES):
    """v3: concat scheme (as v1) with stationary reuse.

    All weights for one population stay SBUF-resident (8MB fp8); the matmul
    loop is m -> half -> kd -> nb so one LDWEIGHTS serves 4 matmuls (one per
    o-block), cutting LDW traffic 4x and keeping the PE stream dense. PSUM
    holds 4 accumulating banks (one per o-block) per m-subtile.
    """
    kt = i_dim // PART
    nb = o_dim // 512
    mb = b // PART
    DR = mybir.MatmulPerfMode.DoubleRow
    nk = kt // 2

    nc = bacc.Bacc("TRN2", target_bir_lowering=False, debug=False,
                   num_devices=n_cores)

    xt_d = nc.dram_tensor("xt", [ppc, PART, kt, b], FP8, kind="ExternalInput")
    w0_d = nc.dram_tensor("w0", [ppc, nb, PART, kt, 512], FP8, kind="ExternalInput")
    w1_d = nc.dram_tensor("w1", [ppc, nb, PART, kt, 512], FP8, kind="ExternalInput")
    out_d = nc.dram_tensor("out", [ppc, b, o_dim], F32, kind="ExternalOutput")

    with tile.TileContext(nc) as tc:
        with (
            tc.tile_pool(name="xpool", bufs=2) as xpool,
            tc.tile_pool(name="wpool", bufs=2 * nb * 2) as wpool,
            tc.tile_pool(name="opool", bufs=6) as opool,
            tc.tile_pool(name="pspool", bufs=8, space="PSUM") as pspool,
        ):
            for pop in range(ppc):
                xt = xpool.tile([PART, kt, b], FP8, tag="xt")
                nxt = xpool.tile([PART, kt, b], FP8, tag="nxt")
                nc.gpsimd.dma_start(out=xt[:], in_=xt_d.ap()[pop])
                nc.vector.tensor_scalar(
                    nxt[:], xt[:], -1.0, 1.0,
                    mybir.AluOpType.mult, mybir.AluOpType.add,
                )
                # all weights for this pop, k-chunked so matmuls start early;
                # w0 on the sync HWDGE ring, w1 on the scalar HWDGE ring
                w0t = [wpool.tile([PART, kt, 512], FP8, tag="w",
                                  name=f"w0t_{pop}_{i}") for i in range(nb)]
                w1t = [wpool.tile([PART, kt, 512], FP8, tag="w",
                                  name=f"w1t_{pop}_{i}") for i in range(nb)]
                for ch in range(0, kt, 4):
                    for nbi in range(nb):
                        nc.sync.dma_start(
                            out=w0t[nbi][:, ch:ch + 4, :],
                            in_=w0_d.ap()[pop, nbi, :, ch:ch + 4, :])
                        nc.scalar.dma_start(
                            out=w1t[nbi][:, ch:ch + 4, :],
                            in_=w1_d.ap()[pop, nbi, :, ch:ch + 4, :])
                for m in range(mb):
                    msl = slice(m * PART, (m + 1) * PART)
                    pss = [pspool.tile([PART, 512], F32, tag="ps",
                                       name=f"ps_{pop}_{m}_{i}") for i in range(nb)]
                    for half, (xsrc, wt) in enumerate(((xt, w0t), (nxt, w1t))):
                        for kd in range(nk):
                            ksl = slice(2 * kd, 2 * kd + 2)
                            for nbi in range(nb):
                                nc.tensor.matmul(
                                    pss[nbi][:], lhsT=xsrc[:, ksl, msl],
                                    rhs=wt[nbi][:, ksl, :],
                                    start=(half == 0 and kd == 0),
                                    stop=(half == 1 and kd == nk - 1),
                                    perf_mode=DR,
                                )
                    for nbi in range(nb):
                        ot = opool.tile([PART, 512], F32)
                        nc.vector.tensor_copy(ot[:], pss[nbi][:])
                        nc.gpsimd.dma_start(
                            out=out_d.ap()[pop, msl, nbi * 512:(nbi + 1) * 512],
                            in_=ot[:],
                        )
    nc.compile()
    return nc


def build_nc_v4(ppc=PPC, b=B, i_dim=I, o_dim=O, n_cores=N_CORES):
    """v4: out = x@(w0-w1) + colsum(w1), wd built by DVE+gpsimd tensor_tensor.

    Halves the PE matmul stream vs the concat scheme (K=2048 instead of 4096).
    Per o-block: load w0/w1, bias = colsum(w1) via an all-ones DR matmul,
    wd = w0-w1 with the k-subtiles split between vector (11) and gpsimd (5)
    engines, main matmuls accumulate x@wd, and the DVE evacuation adds bias
    (tensor_tensor add against a bias tile copied from the bias PSUM bank).
    """
    kt = i_dim // PART
    nb = o_dim // 512
    mb = b // PART
    DR = mybir.MatmulPerfMode.DoubleRow
    nk = kt // 2
    # all subtract work on DVE: offloading 2 k-subtiles to gpsimd measured
    # 128.6us vs 128.0us all-DVE — the DVE's 23us of idle means it is not
    # strictly binding, and the gpsimd offload does not pay
    kdve = kt

    nc = bacc.Bacc("TRN2", target_bir_lowering=False, debug=False,
                   num_devices=n_cores)

    xt_d = nc.dram_tensor("xt", [ppc, PART, kt, b], FP8, kind="ExternalInput")
    w0_d = nc.dram_tensor("w0", [ppc, nb, PART, kt, 512], FP8, kind="ExternalInput")
    w1_d = nc.dram_tensor("w1", [ppc, nb, PART, kt, 512], FP8, kind="ExternalInput")
    out_d = nc.dram_tensor("out", [ppc, b, o_dim], F32, kind="ExternalOutput")

    with tile.TileContext(nc) as tc:
        with (
            tc.tile_pool(name="const", bufs=1) as const,
            tc.tile_pool(name="xpool", bufs=2) as xpool,
            tc.tile_pool(name="wsrc", bufs=6) as wsrc,
            tc.tile_pool(name="wdpool", bufs=4) as wdpool,
            tc.tile_pool(name="bpool", bufs=3) as bpool,
            tc.tile_pool(name="opool", bufs=4) as opool,
            tc.tile_pool(name="pspool", bufs=4, space="PSUM") as pspool,
            tc.tile_pool(name="psbias", bufs=2, space="PSUM") as psbias,
        ):
            ones = const.tile([PART, 2, PART], FP8)
            nc.vector.memset(ones[:], 1.0)
            xts = {}
            state = {}
            blocks = [(pop, nbi) for pop in range(ppc) for nbi in range(nb)]

            def prepare(pop, nbi):
                if nbi == 0:
                    xt = xpool.tile([PART, kt, b], FP8, tag="xt",
                                    name=f"xt_{pop}")
                    xch = min(4, kt)
                    for ch in range(0, kt, xch):
                        nc.scalar.dma_start(
                            out=xt[:, ch:ch + xch, :],
                            in_=xt_d.ap()[pop, :, ch:ch + xch, :])
                    xts[pop] = xt
                w0t = wsrc.tile([PART, kt, 512], FP8, tag="ws",
                                name=f"w0t_{pop}_{nbi}")
                w1t = wsrc.tile([PART, kt, 512], FP8, tag="ws",
                                name=f"w1t_{pop}_{nbi}")
                wch = 2 if (pop == 0 and nbi == 0) else 4
                for ch in range(0, kt, wch):
                    nc.sync.dma_start(
                        out=w1t[:, ch:ch + wch, :],
                        in_=w1_d.ap()[pop, nbi, :, ch:ch + wch, :])
                    nc.scalar.dma_start(
                        out=w0t[:, ch:ch + wch, :],
                        in_=w0_d.ap()[pop, nbi, :, ch:ch + wch, :])
                # bias = colsum(w1) (all rows of psb identical)
                psb = psbias.tile([PART, 512], F32, tag="psb")
                for kd in range(nk):
                    ksl = slice(2 * kd, 2 * kd + 2)
                    nc.tensor.matmul(
                        psb[:], lhsT=ones[:], rhs=w1t[:, ksl, :],
                        start=(kd == 0), stop=(kd == nk - 1), perf_mode=DR)
                bias_sb = bpool.tile([PART, 512], F32, tag="bias")
                nc.vector.tensor_copy(bias_sb[:], psb[:])
                # wd = w0 - w1 on DVE in fine k-chunks; emitted one block
                # AHEAD of the consuming matmuls (software pipeline) so these
                # sit before the previous block's evacuations in the DVE FIFO
                wd = wdpool.tile([PART, kt, 512], FP8, tag="wd")
                sch = max(1, kt // 8)
                for ch in range(0, kdve, sch):
                    nc.vector.tensor_tensor(
                        wd[:, ch:ch + sch, :], w0t[:, ch:ch + sch, :],
                        w1t[:, ch:ch + sch, :], mybir.AluOpType.subtract)
                if kdve < kt:
                    nc.gpsimd.tensor_tensor(
                        wd[:, kdve:, :], w0t[:, kdve:, :], w1t[:, kdve:, :],
                        mybir.AluOpType.subtract)
                state[(pop, nbi)] = (wd, bias_sb)

            def main(pop, nbi):
                wd, bias_sb = state.pop((pop, nbi))
                xt = xts[pop]
                for m in range(mb):
                    ps = pspool.tile([PART, 512], F32, tag="ps",
                                     name=f"ps_{pop}_{nbi}_{m}")
                    msl = slice(m * PART, (m + 1) * PART)
                    for kd in range(nk):
                        ksl = slice(2 * kd, 2 * kd + 2)
                        nc.tensor.matmul(
                            ps[:], lhsT=xt[:, ksl, msl], rhs=wd[:, ksl, :],
                            start=(kd == 0), stop=(kd == nk - 1), perf_mode=DR)
                    ot = opool.tile([PART, 512], F32, tag="ot",
                                    name=f"ot_{pop}_{nbi}_{m}")
                    nc.vector.tensor_tensor(
                        ot[:], ps[:], bias_sb[:], mybir.AluOpType.add)
                    nc.gpsimd.dma_start(
                        out=out_d.ap()[pop, msl, nbi * 512:(nbi + 1) * 512],
                        in_=ot[:])

            for i in range(len(blocks) + 1):
                if i < len(blocks):
                    prepare(*blocks[i])
                if i > 0:
                    main(*blocks[i - 1])
    nc.compile()
    return nc


def build_nc_v5(ppc=PPC, b=B, i_dim=I, o_dim=O, n_cores=N_CORES,
                warmup_mms=12, xor_chunk=4):
    """v5: out = x@wd + colsum(w1), wd built by int32 bitwise-XOR on DVE.

    Key trick: for 0/1 weights cast to fp8e4m3, fp8(w0) XOR fp8(-w1) is
    bit-identical to fp8(w0 - w1) in every case ((1,1) yields 0x80 = -0,
    which accumulates as 0).  The host sends w1n = -w1 (sign folded into
    the cast, +0.0 normalized), so the DVE computes wd with int32 bitwise
    XOR at 4 bytes/lane/cycle -- 4x the fp8 tensor_tensor rate that made
    v4's DVE the rate limiter (99us busy).

    Also: f16 output (exact for integer sums <= 2048, halves store
    traffic vs f32) and a PE warm-up matmul stream at t=0 so the HAM
    clock gate reaches 2.4 GHz before the real matmuls begin.
    """
    kt = i_dim // PART
    nb = o_dim // 512
    mb = b // PART
    DR = mybir.MatmulPerfMode.DoubleRow
    F16 = mybir.dt.float16
    I32 = mybir.dt.int32
    nk = kt // 2

    nc = bacc.Bacc("TRN2", target_bir_lowering=False, debug=False,
                   num_devices=n_cores)

    xt_d = nc.dram_tensor("xt", [ppc, PART, kt, b], FP8, kind="ExternalInput")
    w0_d = nc.dram_tensor("w0", [ppc, nb, PART, kt, 512], FP8, kind="ExternalInput")
    w1_d = nc.dram_tensor("w1", [ppc, nb, PART, kt, 512], FP8, kind="ExternalInput")
    out_d = nc.dram_tensor("out", [ppc, b, o_dim], F16, kind="ExternalOutput")

    with tile.TileContext(nc) as tc:
        with (
            tc.tile_pool(name="const", bufs=1) as const,
            tc.tile_pool(name="xpool", bufs=2) as xpool,
            tc.tile_pool(name="wsrc", bufs=4) as wsrc,
            tc.tile_pool(name="wdpool", bufs=4) as wdpool,
            tc.tile_pool(name="bpool", bufs=3) as bpool,
            tc.tile_pool(name="opool", bufs=6) as opool,
            tc.tile_pool(name="pspool", bufs=4, space="PSUM") as pspool,
            tc.tile_pool(name="psbias", bufs=2, space="PSUM") as psbias,
            tc.tile_pool(name="pswarm", bufs=1, space="PSUM") as pswarm,
        ):
            # --- PE warm-up: dummy matmuls from t~0 keep the PE busy while
            # the first weight DMAs land, so the HAM clock gate is at 8/8
            # (2.4 GHz) when the real stream begins.
            warm = const.tile([PART, 2, 512], FP8)
            nc.scalar.memzero(warm[:])
            psw = pswarm.tile([PART, 512], F32)
            for _ in range(warmup_mms):
                nc.tensor.matmul(psw[:], lhsT=warm[:, :, :PART], rhs=warm[:],
                                 start=True, stop=True, perf_mode=DR)

            ones = const.tile([PART, 2, PART], FP8)
            nc.vector.memset(ones[:], 1.0)
            xts = {}
            state = {}
            blocks = [(pop, nbi) for pop in range(ppc) for nbi in range(nb)]

            def prepare(pop, nbi):
                if nbi == 0:
                    xt = xpool.tile([PART, kt, b], FP8, tag="xt",
                                    name=f"xt_{pop}")
                    xch = min(4, kt)
                    for ch in range(0, kt, xch):
                        nc.scalar.dma_start(
                            out=xt[:, ch:ch + xch, :],
                            in_=xt_d.ap()[pop, :, ch:ch + xch, :])
                    xts[pop] = xt
                # w0 lands directly in the wd tile; w1n in its own tile.
                wd = wdpool.tile([PART, kt, 512], FP8, tag="wd",
                                 name=f"wd_{pop}_{nbi}")
                w1t = wsrc.tile([PART, kt, 512], FP8, tag="ws",
                                name=f"w1t_{pop}_{nbi}")
                wch = 2 if (pop == 0 and nbi == 0) else 4
                for ch in range(0, kt, wch):
                    nc.sync.dma_start(
                        out=w1t[:, ch:ch + wch, :],
                        in_=w1_d.ap()[pop, nbi, :, ch:ch + wch, :])
                    nc.scalar.dma_start(
                        out=wd[:, ch:ch + wch, :],
                        in_=w0_d.ap()[pop, nbi, :, ch:ch + wch, :])
                # -bias = colsum(w1n) via all-ones DR matmul (w1t holds -w1)
                psb = psbias.tile([PART, 512], F32, tag="psb")
                for kd in range(nk):
                    ksl = slice(2 * kd, 2 * kd + 2)
                    nc.tensor.matmul(
                        psb[:], lhsT=ones[:], rhs=w1t[:, ksl, :],
                        start=(kd == 0), stop=(kd == nk - 1), perf_mode=DR)
                # wd = w0 XOR w1n, int32 view: 4 fp8 bytes/lane/cycle.
                # Emitted BEFORE the bias copy so the DVE starts the XOR as
                # soon as the weights land (not serialized behind the bias
                # matmuls' PSUM result).
                for ch in range(0, kt, xor_chunk):
                    csl = slice(ch, ch + xor_chunk)
                    nc.vector.tensor_tensor(
                        wd[:, csl, :].bitcast(I32), wd[:, csl, :].bitcast(I32),
                        w1t[:, csl, :].bitcast(I32), mybir.AluOpType.bitwise_xor)
                bias_sb = bpool.tile([PART, 512], F32, tag="bias")
                nc.vector.tensor_copy(bias_sb[:], psb[:])
                state[(pop, nbi)] = (wd, bias_sb)

            def main(pop, nbi):
                wd, bias_sb = state.pop((pop, nbi))
                xt = xts[pop]
                for m in range(mb):
                    ps = pspool.tile([PART, 512], F32, tag="ps",
                                     name=f"ps_{pop}_{nbi}_{m}")
                    msl = slice(m * PART, (m + 1) * PART)
                    for kd in range(nk):
                        ksl = slice(2 * kd, 2 * kd + 2)
                        nc.tensor.matmul(
                            ps[:], lhsT=xt[:, ksl, msl], rhs=wd[:, ksl, :],
                            start=(kd == 0), stop=(kd == nk - 1), perf_mode=DR)
                    ot = opool.tile([PART, 512], F16, tag="ot",
                                    name=f"ot_{pop}_{nbi}_{m}")
                    # out = psum - (-bias)
                    nc.vector.tensor_tensor(
                        ot[:], ps[:], bias_sb[:], mybir.AluOpType.subtract)
                    # the final block's stores go on the (by now idle) HWDGE
                    # rings: ~0.6us completion vs SWDGE's ~1us + end drain
                    if pop == ppc - 1 and nbi == nb - 1:
                        eng = nc.sync if m % 2 == 0 else nc.scalar
                    else:
                        eng = nc.gpsimd
                    eng.dma_start(
                        out=out_d.ap()[pop, msl, nbi * 512:(nbi + 1) * 512],
                        in_=ot[:])

            for i in range(len(blocks) + 1):
                if i < len(blocks):
                    prepare(*blocks[i])
                if i > 0:
                    main(*blocks[i - 1])
    nc.compile()
    return nc


def build_nc_v9(ppc=PPC, b=B, i_dim=I, o_dim=O, n_cores=N_CORES,
                warmup_mms=8, xor_chunk=4):
    """v9: v5 with a 2-deep bias pipeline.

    PE order [bias0, bias1, main0, bias2, main1, ...]: during the DMA ramp
    the PE runs bias matmuls (which need only w1) instead of idling, and
    each block's w1 deadline moves a block earlier than its w0 deadline.
    Rings: sync = w1 (+ x tails), scalar = x head + w0 -- so w0 (the main
    matmul critical path, via XOR) never queues behind w1 bytes.
    Last block's stores go on the by-then-idle HWDGE rings.
    """
    kt = i_dim // PART
    nb = o_dim // 512
    mb = b // PART
    DR = mybir.MatmulPerfMode.DoubleRow
    F16 = mybir.dt.float16
    I32 = mybir.dt.int32
    nk = kt // 2

    nc = bacc.Bacc("TRN2", target_bir_lowering=False, debug=False,
                   num_devices=n_cores)

    xt_d = nc.dram_tensor("xt", [ppc, PART, kt, b], FP8, kind="ExternalInput")
    w0_d = nc.dram_tensor("w0", [ppc, nb, PART, kt, 512], FP8, kind="ExternalInput")
    w1_d = nc.dram_tensor("w1", [ppc, nb, PART, kt, 512], FP8, kind="ExternalInput")
    out_d = nc.dram_tensor("out", [ppc, b, o_dim], F16, kind="ExternalOutput")

    with tile.TileContext(nc) as tc:
        with (
            tc.tile_pool(name="const", bufs=1) as const,
            tc.tile_pool(name="xpool", bufs=2) as xpool,
            tc.tile_pool(name="wsrc", bufs=4) as wsrc,
            tc.tile_pool(name="wdpool", bufs=4) as wdpool,
            tc.tile_pool(name="bpool", bufs=3) as bpool,
            tc.tile_pool(name="opool", bufs=6) as opool,
            tc.tile_pool(name="pspool", bufs=4, space="PSUM") as pspool,
            tc.tile_pool(name="psbias", bufs=2, space="PSUM") as psbias,
            tc.tile_pool(name="pswarm", bufs=1, space="PSUM") as pswarm,
        ):
            warm = const.tile([PART, 2, 512], FP8)
            nc.scalar.memzero(warm[:])
            psw = pswarm.tile([PART, 512], F32)
            for _ in range(warmup_mms):
                nc.tensor.matmul(psw[:], lhsT=warm[:, :, :PART], rhs=warm[:],
                                 start=True, stop=True, perf_mode=DR)

            ones = const.tile([PART, 2, PART], FP8)
            nc.vector.memset(ones[:], 1.0)
            xts = {}
            state = {}
            blocks = [(pop, nbi) for pop in range(ppc) for nbi in range(nb)]
            nblocks = len(blocks)

            # x(0)'s first chunk leads the scalar ring (main(0) stationary)
            xt0 = xpool.tile([PART, kt, b], FP8, tag="xt", name="xt_0")
            xts[0] = xt0
            nc.scalar.dma_start(out=xt0[:, 0:4, :], in_=xt_d.ap()[0, :, 0:4, :])

            def prep_w1bias(i):
                pop, nbi = blocks[i]
                w1t = wsrc.tile([PART, kt, 512], FP8, tag="ws",
                                name=f"w1t_{pop}_{nbi}")
                wch = 2 if i == 0 else 4
                for ch in range(0, kt, wch):
                    nc.sync.dma_start(
                        out=w1t[:, ch:ch + wch, :],
                        in_=w1_d.ap()[pop, nbi, :, ch:ch + wch, :])
                if i == 1:   # x(0) tail on the w1 ring
                    nc.sync.dma_start(out=xt0[:, 4:kt, :],
                                      in_=xt_d.ap()[0, :, 4:kt, :])
                if i == 4 and ppc > 1:   # x(1) on the w1 ring
                    xt1 = xpool.tile([PART, kt, b], FP8, tag="xt", name="xt_1")
                    xts[1] = xt1
                    nc.sync.dma_start(out=xt1[:], in_=xt_d.ap()[1])
                # -bias = colsum(w1n) via all-ones DR matmul
                psb = psbias.tile([PART, 512], F32, tag="psb")
                for kd in range(nk):
                    ksl = slice(2 * kd, 2 * kd + 2)
                    nc.tensor.matmul(
                        psb[:], lhsT=ones[:], rhs=w1t[:, ksl, :],
                        start=(kd == 0), stop=(kd == nk - 1), perf_mode=DR)
                bias_sb = bpool.tile([PART, 512], F32, tag="bias")
                nc.vector.tensor_copy(bias_sb[:], psb[:])
                state[i] = (w1t, bias_sb)

            def prep_w0xor(i):
                pop, nbi = blocks[i]
                w1t, bias_sb = state[i]
                wd = wdpool.tile([PART, kt, 512], FP8, tag="wd",
                                 name=f"wd_{pop}_{nbi}")
                wch = 2 if i == 0 else 4
                for ch in range(0, kt, wch):
                    nc.scalar.dma_start(
                        out=wd[:, ch:ch + wch, :],
                        in_=w0_d.ap()[pop, nbi, :, ch:ch + wch, :])
                # wd = w0 XOR w1n (int32 view, 4 fp8 bytes/lane/cycle)
                for ch in range(0, kt, xor_chunk):
                    csl = slice(ch, ch + xor_chunk)
                    nc.vector.tensor_tensor(
                        wd[:, csl, :].bitcast(I32), wd[:, csl, :].bitcast(I32),
                        w1t[:, csl, :].bitcast(I32), mybir.AluOpType.bitwise_xor)
                state[i] = (wd, bias_sb)

            def main(i):
                pop, nbi = blocks[i]
                wd, bias_sb = state.pop(i)
                xt = xts[pop]
                for m in range(mb):
                    ps = pspool.tile([PART, 512], F32, tag="ps",
                                     name=f"ps_{pop}_{nbi}_{m}")
                    msl = slice(m * PART, (m + 1) * PART)
                    for kd in range(nk):
                        ksl = slice(2 * kd, 2 * kd + 2)
                        nc.tensor.matmul(
                            ps[:], lhsT=xt[:, ksl, msl], rhs=wd[:, ksl, :],
                            start=(kd == 0), stop=(kd == nk - 1), perf_mode=DR)
                    ot = opool.tile([PART, 512], F16, tag="ot",
                                    name=f"ot_{pop}_{nbi}_{m}")
                    # out = psum - (-bias)
                    nc.vector.tensor_tensor(
                        ot[:], ps[:], bias_sb[:], mybir.AluOpType.subtract)
                    if i == nblocks - 1:
                        eng = nc.sync if m % 2 == 0 else nc.scalar
                    else:
                        eng = nc.gpsimd
                    eng.dma_start(
                        out=out_d.ap()[pop, msl, nbi * 512:(nbi + 1) * 512],
                        in_=ot[:])

            prep_w1bias(0)
            prep_w0xor(0)
            prep_w1bias(1)
            for i in range(1, nblocks):
                main(i - 1)
                if i + 1 < nblocks:
                    prep_w1bias(i + 1)
                prep_w0xor(i)
            main(nblocks - 1)
    nc.compile()
    return nc


def build_nc_v7(ppc=PPC, b=B, i_dim=I, o_dim=O, n_cores=N_CORES,
                warmup_mms=12, xor_chunk=4):
    """v7: v5 structure (one-ahead prepare, lookahead-1 DMA) plus:
      - XOR emitted before the bias PSUM copy in the DVE queue, so it
        starts as soon as the weights land instead of serializing behind
        the bias matmuls' result;
      - pop 1's x loaded one block earlier (v5 stalled 3us on it);
      - the last two blocks' stores go on the by-then-idle HWDGE rings,
        avoiding the multi-us SWDGE drain after the final matmul.
    """
    kt = i_dim // PART
    nb = o_dim // 512
    mb = b // PART
    DR = mybir.MatmulPerfMode.DoubleRow
    F16 = mybir.dt.float16
    I32 = mybir.dt.int32
    nk = kt // 2
    nblocks = ppc * nb

    nc = bacc.Bacc("TRN2", target_bir_lowering=False, debug=False,
                   num_devices=n_cores)

    xt_d = nc.dram_tensor("xt", [ppc, PART, kt, b], FP8, kind="ExternalInput")
    w0_d = nc.dram_tensor("w0", [ppc, nb, PART, kt, 512], FP8, kind="ExternalInput")
    w1_d = nc.dram_tensor("w1", [ppc, nb, PART, kt, 512], FP8, kind="ExternalInput")
    out_d = nc.dram_tensor("out", [ppc, b, o_dim], F16, kind="ExternalOutput")

    with tile.TileContext(nc) as tc:
        with (
            tc.tile_pool(name="const", bufs=1) as const,
            tc.tile_pool(name="xpool", bufs=2) as xpool,
            tc.tile_pool(name="wsrc", bufs=4) as wsrc,
            tc.tile_pool(name="wdpool", bufs=4) as wdpool,
            tc.tile_pool(name="bpool", bufs=3) as bpool,
            tc.tile_pool(name="opool", bufs=6) as opool,
            tc.tile_pool(name="pspool", bufs=4, space="PSUM") as pspool,
            tc.tile_pool(name="psbias", bufs=2, space="PSUM") as psbias,
            tc.tile_pool(name="pswarm", bufs=1, space="PSUM") as pswarm,
        ):
            warm = const.tile([PART, 2, 512], FP8)
            nc.scalar.memzero(warm[:])
            psw = pswarm.tile([PART, 512], F32)
            for _ in range(warmup_mms):
                nc.tensor.matmul(psw[:], lhsT=warm[:, :, :PART], rhs=warm[:],
                                 start=True, stop=True, perf_mode=DR)

            ones = const.tile([PART, 2, PART], FP8)
            nc.vector.memset(ones[:], 1.0)
            xts = {}
            state = {}
            blocks = [(pop, nbi) for pop in range(ppc) for nbi in range(nb)]

            def load_x(pop):
                xt = xpool.tile([PART, kt, b], FP8, tag="xt",
                                name=f"xt_{pop}")
                xts[pop] = xt
                xch = min(4, kt)
                for ch in range(0, kt, xch):
                    nc.scalar.dma_start(
                        out=xt[:, ch:ch + xch, :],
                        in_=xt_d.ap()[pop, :, ch:ch + xch, :])

            def prepare(pop, nbi):
                if pop == 0 and nbi == 0:
                    load_x(0)
                wd = wdpool.tile([PART, kt, 512], FP8, tag="wd",
                                 name=f"wd_{pop}_{nbi}")
                w1t = wsrc.tile([PART, kt, 512], FP8, tag="ws",
                                name=f"w1t_{pop}_{nbi}")
                wch = 2 if (pop == 0 and nbi == 0) else 4
                for ch in range(0, kt, wch):
                    nc.sync.dma_start(
                        out=w1t[:, ch:ch + wch, :],
                        in_=w1_d.ap()[pop, nbi, :, ch:ch + wch, :])
                    nc.scalar.dma_start(
                        out=wd[:, ch:ch + wch, :],
                        in_=w0_d.ap()[pop, nbi, :, ch:ch + wch, :])
                if nbi == 3 and pop + 1 < ppc:
                    load_x(pop + 1)
                # -bias = colsum(w1n) via all-ones DR matmul
                psb = psbias.tile([PART, 512], F32, tag="psb")
                for kd in range(nk):
                    ksl = slice(2 * kd, 2 * kd + 2)
                    nc.tensor.matmul(
                        psb[:], lhsT=ones[:], rhs=w1t[:, ksl, :],
                        start=(kd == 0), stop=(kd == nk - 1), perf_mode=DR)
                bias_sb = bpool.tile([PART, 512], F32, tag="bias")
                nc.vector.tensor_copy(bias_sb[:], psb[:])
                state[(pop, nbi)] = (wd, w1t, bias_sb)

            def prep_xor(pop, nbi):
                # wd = w0 XOR w1n (int32 view, 4 fp8 bytes/lane/cycle).
                # Emitted AFTER main(i-1)'s evacuations in the DVE queue: a
                # DMA-gated op ahead of the evacs would back up PSUM and
                # stall the PE even when main(i-1)'s own data is ready.
                wd, w1t, bias_sb = state[(pop, nbi)]
                for ch in range(0, kt, xor_chunk):
                    csl = slice(ch, ch + xor_chunk)
                    nc.vector.tensor_tensor(
                        wd[:, csl, :].bitcast(I32), wd[:, csl, :].bitcast(I32),
                        w1t[:, csl, :].bitcast(I32), mybir.AluOpType.bitwise_xor)
                state[(pop, nbi)] = (wd, bias_sb)

            def main(pop, nbi):
                wd, bias_sb = state.pop((pop, nbi))
                xt = xts[pop]
                blk_i = pop * nb + nbi
                for m in range(mb):
                    ps = pspool.tile([PART, 512], F32, tag="ps",
                                     name=f"ps_{pop}_{nbi}_{m}")
                    msl = slice(m * PART, (m + 1) * PART)
                    for kd in range(nk):
                        ksl = slice(2 * kd, 2 * kd + 2)
                        nc.tensor.matmul(
                            ps[:], lhsT=xt[:, ksl, msl], rhs=wd[:, ksl, :],
                            start=(kd == 0), stop=(kd == nk - 1), perf_mode=DR)
                    ot = opool.tile([PART, 512], F16, tag="ot",
                                    name=f"ot_{pop}_{nbi}_{m}")
                    # out = psum - (-bias)
                    nc.vector.tensor_tensor(
                        ot[:], ps[:], bias_sb[:], mybir.AluOpType.subtract)
                    eng = nc.gpsimd
                    eng.dma_start(
                        out=out_d.ap()[pop, msl, nbi * 512:(nbi + 1) * 512],
                        in_=ot[:])

            for i in range(len(blocks) + 1):
                if i < len(blocks):
                    prepare(*blocks[i])
                if i > 0:
                    main(*blocks[i - 1])
                if i < len(blocks):
                    prep_xor(*blocks[i])
    nc.compile()
    return nc


def build_nc_v6(ppc=PPC, b=B, i_dim=I, o_dim=O, n_cores=N_CORES,
                warmup_mms=3, xor_chunk=4, lookahead=8, wch_steady=4,
                late_store_from=6):
    """v6: v5 with decoupled DMA lookahead.

    dma_block() emits only DMA traffic and runs `lookahead` blocks ahead
    of the PE/DVE stream, so HBM prefetch never falls behind the PE
    (v5's 15-40us stall cluster).  Block ordering on the scalar ring puts
    w0(0) before the bulk of x so the first XOR can start early; x's
    first chunk goes ahead of everything so main(0)'s stationary is
    ready.  Bias PSUM->SBUF copies move to the scalar engine (ACT is
    close to PSUM; DVE keeps only XOR + evacuation).
    """
    kt = i_dim // PART
    nb = o_dim // 512
    mb = b // PART
    DR = mybir.MatmulPerfMode.DoubleRow
    F16 = mybir.dt.float16
    I32 = mybir.dt.int32
    nk = kt // 2

    nc = bacc.Bacc("TRN2", target_bir_lowering=False, debug=False,
                   num_devices=n_cores)

    xt_d = nc.dram_tensor("xt", [ppc, PART, kt, b], FP8, kind="ExternalInput")
    w0_d = nc.dram_tensor("w0", [ppc, nb, PART, kt, 512], FP8, kind="ExternalInput")
    w1_d = nc.dram_tensor("w1", [ppc, nb, PART, kt, 512], FP8, kind="ExternalInput")
    out_d = nc.dram_tensor("out", [ppc, b, o_dim], F16, kind="ExternalOutput")

    with tile.TileContext(nc) as tc:
        with (
            tc.tile_pool(name="const", bufs=1) as const,
            tc.tile_pool(name="xpool", bufs=2) as xpool,
            tc.tile_pool(name="wsrc", bufs=min(lookahead + 2, 8)) as wsrc,
            tc.tile_pool(name="wdpool", bufs=min(lookahead + 2, 8)) as wdpool,
            tc.tile_pool(name="bpool", bufs=3) as bpool,
            tc.tile_pool(name="opool", bufs=6) as opool,
            tc.tile_pool(name="pspool", bufs=4, space="PSUM") as pspool,
            tc.tile_pool(name="psbias", bufs=2, space="PSUM") as psbias,
            tc.tile_pool(name="pswarm", bufs=1, space="PSUM") as pswarm,
        ):
            warm = const.tile([PART, 2, 512], FP8)
            nc.scalar.memzero(warm[:])
            psw = pswarm.tile([PART, 512], F32)
            for _ in range(warmup_mms):
                nc.tensor.matmul(psw[:], lhsT=warm[:, :, :PART], rhs=warm[:],
                                 start=True, stop=True, perf_mode=DR)

            ones = const.tile([PART, 2, PART], FP8)
            nc.vector.memset(ones[:], 1.0)
            xts = {}
            dstate = {}
            state = {}
            blocks = [(pop, nbi) for pop in range(ppc) for nbi in range(nb)]

            def load_x(pop):
                # split across both HWDGE rings to keep them balanced
                xt = xpool.tile([PART, kt, b], FP8, tag="xt",
                                name=f"xt_{pop}")
                xts[pop] = xt
                h = kt // 2
                nc.sync.dma_start(out=xt[:, 0:h, :],
                                  in_=xt_d.ap()[pop, :, 0:h, :])
                nc.scalar.dma_start(out=xt[:, h:kt, :],
                                    in_=xt_d.ap()[pop, :, h:kt, :])

            def dma_block(pop, nbi):
                first = (pop == 0 and nbi == 0)
                if first:
                    # first x chunk ahead of everything: main(0)'s stationary
                    xt = xpool.tile([PART, kt, b], FP8, tag="xt", name="xt_0")
                    xts[0] = xt
                    nc.scalar.dma_start(out=xt[:, 0:4, :],
                                        in_=xt_d.ap()[0, :, 0:4, :])
                wd = wdpool.tile([PART, kt, 512], FP8, tag="wd",
                                 name=f"wd_{pop}_{nbi}")
                w1t = wsrc.tile([PART, kt, 512], FP8, tag="ws",
                                name=f"w1t_{pop}_{nbi}")
                # chunk-interleave each tensor across BOTH rings so neither
                # ring ever carries more than half of any block's bytes --
                # the queues get equal SDMA service, so an imbalanced ring
                # directly delays its tensors (v6b regression)
                wch = 2 if first else wch_steady
                for j, ch in enumerate(range(0, kt, wch)):
                    e0, e1 = (nc.sync, nc.scalar) if j % 2 == 0 else                              (nc.scalar, nc.sync)
                    e0.dma_start(
                        out=w1t[:, ch:ch + wch, :],
                        in_=w1_d.ap()[pop, nbi, :, ch:ch + wch, :])
                    e1.dma_start(
                        out=wd[:, ch:ch + wch, :],
                        in_=w0_d.ap()[pop, nbi, :, ch:ch + wch, :])
                if first:
                    xt = xts[0]
                    nc.sync.dma_start(out=xt[:, 4:10, :],
                                      in_=xt_d.ap()[0, :, 4:10, :])
                    nc.scalar.dma_start(out=xt[:, 10:kt, :],
                                        in_=xt_d.ap()[0, :, 10:kt, :])
                elif nbi == 2 and pop + 1 < ppc:
                    # next pop's x after this block's weights: lands well
                    # before block (pop+1, 0) needs it
                    load_x(pop + 1)
                dstate[(pop, nbi)] = (wd, w1t)

            def pe_xor(pop, nbi):
                # wd = w0 XOR w1n, int32 view: 4 fp8 bytes/lane/cycle.
                # Emitted a full block ahead of the consuming matmuls, and
                # ahead of the previous block's evacuations in the DVE queue,
                # so it runs as soon as the weights land.
                wd, w1t = dstate[(pop, nbi)]
                for ch in range(0, kt, xor_chunk):
                    csl = slice(ch, ch + xor_chunk)
                    nc.vector.tensor_tensor(
                        wd[:, csl, :].bitcast(I32), wd[:, csl, :].bitcast(I32),
                        w1t[:, csl, :].bitcast(I32), mybir.AluOpType.bitwise_xor)

            def pe_bias(pop, nbi):
                # -bias = colsum(w1n) via all-ones DR matmul.  Emitted AFTER
                # main(i-1) so block i's w1 DMA deadline is a full block
                # later than the main matmuls that consume wd(i).
                wd, w1t = dstate.pop((pop, nbi))
                psb = psbias.tile([PART, 512], F32, tag="psb")
                for kd in range(nk):
                    ksl = slice(2 * kd, 2 * kd + 2)
                    nc.tensor.matmul(
                        psb[:], lhsT=ones[:], rhs=w1t[:, ksl, :],
                        start=(kd == 0), stop=(kd == nk - 1), perf_mode=DR)
                bias_sb = bpool.tile([PART, 512], F32, tag="bias")
                nc.vector.tensor_copy(bias_sb[:], psb[:])
                state[(pop, nbi)] = (wd, bias_sb)

            def main(pop, nbi):
                wd, bias_sb = state.pop((pop, nbi))
                xt = xts[pop]
                for m in range(mb):
                    ps = pspool.tile([PART, 512], F32, tag="ps",
                                     name=f"ps_{pop}_{nbi}_{m}")
                    msl = slice(m * PART, (m + 1) * PART)
                    for kd in range(nk):
                        ksl = slice(2 * kd, 2 * kd + 2)
                        nc.tensor.matmul(
                            ps[:], lhsT=xt[:, ksl, msl], rhs=wd[:, ksl, :],
                            start=(kd == 0), stop=(kd == nk - 1), perf_mode=DR)
                    ot = opool.tile([PART, 512], F16, tag="ot",
                                    name=f"ot_{pop}_{nbi}_{m}")
                    # out = psum - (-bias)
                    nc.vector.tensor_tensor(
                        ot[:], ps[:], bias_sb[:], mybir.AluOpType.subtract)
                    # late blocks store on the HWDGE rings (idle once the
                    # loads finish): avoids the multi-us SWDGE drain after
                    # the final matmul
                    blk_i = pop * nb + nbi
                    if blk_i >= late_store_from:
                        eng = nc.sync if m % 2 == 0 else nc.scalar
                    else:
                        eng = nc.gpsimd
                    eng.dma_start(
                        out=out_d.ap()[pop, msl, nbi * 512:(nbi + 1) * 512],
                        in_=ot[:])

            for i in range(min(lookahead, len(blocks))):
                dma_block(*blocks[i])
            # software pipeline, per iteration i:
            #   xor(i)     DVE -- before main(i-1)'s evacs in the DVE queue
            #   main(i-1)  PE stream + evac + store
            #   bias(i)    PE -- after main(i-1), relaxing w1(i)'s deadline
            for i in range(len(blocks) + 1):
                if i < len(blocks):
                    pe_xor(*blocks[i])
                    if i + lookahead < len(blocks):
                        dma_block(*blocks[i + lookahead])
                if i > 0:
                    main(*blocks[i - 1])
                if i < len(blocks):
                    pe_bias(*blocks[i])
    nc.compile()
    return nc


def build_nc_v10(ppc=PPC, b=B, i_dim=I, o_dim=O, n_cores=N_CORES,
                 warmup_mms=8, xor_chunk=2, xor_eng="vector",
                 psb_eng="vector", late_store_from=99, tree_eng="vector",
                 tree_split=0):
    """v10: full-prefetch + bias colsum off the PE + kd-outer main loop.

    Three structural changes vs v5/v7 (103-108us):
      1. ALL loads (x, w0, w1: 20.9MB/core) are issued up front in global
         deadline order, each tensor split half/half across the two HWDGE
         rings so both rings carry identical byte streams.  SBUF holds every
         weight tile (64KB/partition); the rings never idle and there is no
         per-block dependency stall on prefetch.
      2. bias = colsum(w1n) no longer streams all of w1 through the PE
         (8 DR MMs/block = 13.8us/core).  A single DVE tensor_tensor adds
         adjacent k-subtile pairs (w1 tile shaped [128, 8, 2, 512], exact in
         fp8: sums in [-2, 0]), then 4 short DR MMs reduce the 8 partials.
         PE bias cost drops 8.6us/core; DVE absorbs 2.9us/block.
      3. Main matmuls run kd-outer / m-inner over 4 concurrent PSUM banks,
         so each wd chunk is fully consumed as it lands: after the last
         weight byte of the kernel only ~4 matmuls remain (was ~25).
    Also: warm-up via vector.memset (scalar.memzero dragged in a 1.3us
    ACT_TABLE_LOAD before the first warm matmul), and the last block's
    stores ride the by-then-idle HWDGE rings.
    """
    kt = i_dim // PART          # 16
    nb = o_dim // 512           # 4
    mb = b // PART              # 4
    DR = mybir.MatmulPerfMode.DoubleRow
    F16 = mybir.dt.float16
    I32 = mybir.dt.int32
    nk = kt // 2                # 8 DR matmuls per (m, block)
    kh = kt // 2                # pair-groups per weight tile (8)
    nblocks = ppc * nb

    nc = bacc.Bacc("TRN2", target_bir_lowering=False, debug=False,
                   num_devices=n_cores)

    xt_d = nc.dram_tensor("xt", [ppc, PART, kt, b], FP8, kind="ExternalInput")
    # same bytes as [ppc, nb, 128, kt, 512]; the [kh, 2] split exposes
    # adjacent-pair adds as one multi-dim AP tensor_tensor
    w0_d = nc.dram_tensor("w0", [ppc, nb, PART, kh, 2, 512], FP8,
                          kind="ExternalInput")
    w1_d = nc.dram_tensor("w1", [ppc, nb, PART, kh, 2, 512], FP8,
                          kind="ExternalInput")
    out_d = nc.dram_tensor("out", [ppc, b, o_dim], F16, kind="ExternalOutput")

    with tile.TileContext(nc) as tc:
        with (
            tc.tile_pool(name="const", bufs=1) as const,
            tc.tile_pool(name="xpool", bufs=2) as xpool,
            tc.tile_pool(name="wsrc", bufs=nblocks) as wsrc,
            tc.tile_pool(name="wdpool", bufs=nblocks) as wdpool,
            tc.tile_pool(name="s8pool", bufs=3) as s8pool,
            tc.tile_pool(name="bpool", bufs=3) as bpool,
            tc.tile_pool(name="opool", bufs=12) as opool,
            tc.tile_pool(name="pspool", bufs=4, space="PSUM") as pspool,
            tc.tile_pool(name="psbias", bufs=2, space="PSUM") as psbias,
            tc.tile_pool(name="pswarm", bufs=1, space="PSUM") as pswarm,
        ):
            # PE warm-up from t~0 (vector memset: no ACT table load)
            warm = const.tile([PART, 2, 512], FP8)
            nc.vector.memset(warm[:], 0.0)
            ones = const.tile([PART, 2, PART], FP8)
            nc.vector.memset(ones[:], 1.0)
            psw = pswarm.tile([PART, 512], F32)
            for _ in range(warmup_mms):
                nc.tensor.matmul(psw[:], lhsT=warm[:, :, :PART], rhs=warm[:],
                                 start=True, stop=True, perf_mode=DR)

            blocks = [(pop, nbi) for pop in range(ppc) for nbi in range(nb)]

            # ---- all loads up front, deadline order, half per HWDGE ring.
            # Block 0 lands in fine chunks so the PE can chase it; the rest
            # are single 512KB halves (best SDMA efficiency).
            xts = [xpool.tile([PART, kt, b], FP8, tag="xt", name=f"xt_{p}")
                   for p in range(ppc)]
            w1t = [wsrc.tile([PART, kh, 2, 512], FP8, tag="ws",
                             name=f"w1t_{i}") for i in range(nblocks)]
            wd4 = [wdpool.tile([PART, kh, 2, 512], FP8, tag="wd",
                               name=f"wd_{i}") for i in range(nblocks)]

            def load_w(i, chunks):
                pop, nbi = blocks[i]
                c0 = 0
                for ch in chunks:   # ch = number of kh pair-groups
                    h = ch // 2
                    nc.sync.dma_start(
                        out=w1t[i][:, c0:c0 + h, :, :],
                        in_=w1_d.ap()[pop, nbi, :, c0:c0 + h, :, :])
                    nc.scalar.dma_start(
                        out=w1t[i][:, c0 + h:c0 + ch, :, :],
                        in_=w1_d.ap()[pop, nbi, :, c0 + h:c0 + ch, :, :])
                    nc.sync.dma_start(
                        out=wd4[i][:, c0:c0 + h, :, :],
                        in_=w0_d.ap()[pop, nbi, :, c0:c0 + h, :, :])
                    nc.scalar.dma_start(
                        out=wd4[i][:, c0 + h:c0 + ch, :, :],
                        in_=w0_d.ap()[pop, nbi, :, c0 + h:c0 + ch, :, :])
                    c0 += ch

            def load_x(p, k0, k1):
                h = (k0 + k1) // 2
                nc.sync.dma_start(out=xts[p][:, k0:h, :],
                                  in_=xt_d.ap()[p, :, k0:h, :])
                nc.scalar.dma_start(out=xts[p][:, h:k1, :],
                                    in_=xt_d.ap()[p, :, h:k1, :])

            load_w(0, [2, 2, 4])    # w1/w0 block 0: {2,2,4} pair-groups
            load_x(0, 0, 4)         # x0 head: subtiles 0-3
            load_x(0, 4, 10)
            load_x(0, 10, kt)
            load_w(1, [4, 4])
            for i in range(2, nblocks):
                if i == 4 and ppc > 1:
                    load_x(1, 0, kt)
                load_w(i, [kh])

            # ---- per-block compute chain
            state = {}

            def prep_bias_xor(i):
                pop, nbi = blocks[i]
                # s8[j] = w1n[2j] + w1n[2j+1]  (fp8-exact: values in [-2,0])
                s8 = s8pool.tile([PART, kh, 512], FP8, tag="s8",
                                 name=f"s8_{i}")
                teng = {"vector": nc.vector, "gpsimd": nc.gpsimd}[tree_eng]
                if i == 0:
                    for c0, c1 in ((0, 2), (2, 4), (4, kh)):
                        teng.tensor_tensor(
                            s8[:, c0:c1, :], w1t[i][:, c0:c1, 0, :],
                            w1t[i][:, c0:c1, 1, :], mybir.AluOpType.add)
                elif tree_split:
                    h = tree_split
                    nc.gpsimd.tensor_tensor(
                        s8[:, :h, :], w1t[i][:, :h, 0, :],
                        w1t[i][:, :h, 1, :], mybir.AluOpType.add)
                    nc.vector.tensor_tensor(
                        s8[:, h:, :], w1t[i][:, h:, 0, :],
                        w1t[i][:, h:, 1, :], mybir.AluOpType.add)
                else:
                    teng.tensor_tensor(
                        s8[:], w1t[i][:, :, 0, :], w1t[i][:, :, 1, :],
                        mybir.AluOpType.add)
                # -bias = colsum(s8) via 4 short DR matmuls
                psb = psbias.tile([PART, 512], F32, tag="psb",
                                  name=f"psb_{i}")
                for j in range(kh // 2):
                    nc.tensor.matmul(
                        psb[:], lhsT=ones[:], rhs=s8[:, 2 * j:2 * j + 2, :],
                        start=(j == 0), stop=(j == kh // 2 - 1), perf_mode=DR)
                bias_sb = bpool.tile([PART, 512], F32, tag="bias",
                                     name=f"bias_{i}")
                if psb_eng == "scalar":
                    nc.scalar.copy(bias_sb[:], psb[:])
                else:
                    nc.vector.tensor_copy(bias_sb[:], psb[:])
                # wd = w0 XOR w1n (int32 view): fp8(w0) ^ fp8(-w1) is
                # bit-identical to fp8(w0-w1) for 0/1 weights
                xeng = {"vector": nc.vector, "gpsimd": nc.gpsimd}[xor_eng]
                for c in range(0, kh, xor_chunk):
                    csl = slice(c, c + xor_chunk)
                    xeng.tensor_tensor(
                        wd4[i][:, csl, :, :].bitcast(I32),
                        wd4[i][:, csl, :, :].bitcast(I32),
                        w1t[i][:, csl, :, :].bitcast(I32),
                        mybir.AluOpType.bitwise_xor)
                state[i] = bias_sb

            def main(i):
                pop, nbi = blocks[i]
                bias_sb = state.pop(i)
                xt = xts[pop]
                wd = wd4[i]
                pss = [pspool.tile([PART, 512], F32, tag="ps",
                                   name=f"ps_{i}_{m}") for m in range(mb)]
                osl = slice(nbi * 512, (nbi + 1) * 512)
                for kd in range(nk):
                    for m in range(mb):
                        msl = slice(m * PART, (m + 1) * PART)
                        nc.tensor.matmul(
                            pss[m][:], lhsT=xt[:, 2 * kd:2 * kd + 2, msl],
                            rhs=wd[:, kd, :, :],
                            start=(kd == 0), stop=(kd == nk - 1),
                            perf_mode=DR)
                        if kd == nk - 1:
                            # evac chases the stops; bank m is free again
                            # ~3 matmuls later for the next block
                            ot = opool.tile([PART, 512], F16, tag="ot",
                                            name=f"ot_{i}_{m}")
                            nc.vector.tensor_tensor(
                                ot[:], pss[m][:], bias_sb[:],
                                mybir.AluOpType.subtract)
                            if i >= late_store_from or i == nblocks - 1:
                                eng = nc.sync if m % 2 == 0 else nc.scalar
                            else:
                                eng = nc.gpsimd
                            eng.dma_start(out=out_d.ap()[pop, msl, osl],
                                          in_=ot[:])

            prep_bias_xor(0)
            for i in range(nblocks):
                main(i)
                if i + 1 < nblocks:
                    prep_bias_xor(i + 1)
    nc.compile()
    return nc


def build_nc_v12(ppc=PPC, b=B, i_dim=I, o_dim=O, n_cores=N_CORES,
                 warmup_mms=4, bit_dtype="int16", late_store_from=5,
                 tree_pairs=8):
    """v12: v10 structure with the DVE/ACT/PE work rebalanced.

    (The scaled {0,+-128} all-bitwise tree was tried and is mathematically
    dead: this fp8 is IEEE e4m3, max 240, and the byte trick inherently
    lands on exp=1111 = inf.  Bytes were verified identical to the fp8-add
    path on device, so int16 bitwise TT + custom-imm STT do work on DVE.)

    vs v10:
      - bias pair-tree: one fp8 TT add per block on DVE (3.57us measured);
        bias finals are 4 short DR MMs.
      - XOR runs on int16 views (2-byte dtype qualifies for the DVE 2x
        packed perf mode; int32 gets none).
      - psb -> bias_sb copy moves to ACT.
      - Block 0's bias streams w1 through the PE directly (raw v5-style
        MMs double as clock warm-up while block 0 prefetches).
      - All weight loads on the sync ring in strict deadline order (a
        single HWDGE queue sustains ~425 GB/s); x loads ride scalar.
      - Last block runs m-outer so only one evac+store trails the last MM.
    """
    kt = i_dim // PART
    nb = o_dim // 512
    mb = b // PART
    DR = mybir.MatmulPerfMode.DoubleRow
    F16 = mybir.dt.float16
    BIT = {"int16": mybir.dt.int16, "int32": mybir.dt.int32}[bit_dtype]
    SHIFT_OR = mybir.AluOpType.logical_shift_right
    nk = kt // 2
    kh = kt // 2
    nblocks = ppc * nb

    nc = bacc.Bacc("TRN2", target_bir_lowering=False, debug=False,
                   num_devices=n_cores)

    # NOTE: an m-major x (per-m [PART, kt, 128] tiles, contiguous lhsT)
    # was tried and SLOWED every matmul 20% (216->259ns inter-MM): the
    # strided lhsT slice xt[:, ksl, msl] from one monolithic tile is the
    # fast path on this PE.  Keep x k-subtile-major.
    xt_d = nc.dram_tensor("xt", [ppc, PART, kt, b], FP8,
                          kind="ExternalInput")
    w0_d = nc.dram_tensor("w0", [ppc, nb, PART, kh, 2, 512], FP8,
                          kind="ExternalInput")
    w1_d = nc.dram_tensor("w1", [ppc, nb, PART, kh, 2, 512], FP8,
                          kind="ExternalInput")
    out_d = nc.dram_tensor("out", [ppc, b, o_dim], F16, kind="ExternalOutput")

    with tile.TileContext(nc) as tc:
        with (
            tc.tile_pool(name="const", bufs=1) as const,
            tc.tile_pool(name="xpool", bufs=2) as xpool,
            tc.tile_pool(name="wsrc", bufs=nblocks) as wsrc,
            tc.tile_pool(name="wdpool", bufs=nblocks) as wdpool,
            tc.tile_pool(name="s8pool", bufs=3) as s8pool,
            tc.tile_pool(name="bpool", bufs=3) as bpool,
            tc.tile_pool(name="opool", bufs=12) as opool,
            tc.tile_pool(name="pspool", bufs=4, space="PSUM") as pspool,
            tc.tile_pool(name="psbias", bufs=2, space="PSUM") as psbias,
            tc.tile_pool(name="pswarm", bufs=1, space="PSUM") as pswarm,
        ):
            warm = const.tile([PART, 2, 512], FP8)
            nc.vector.memset(warm[:], 0.0)
            ones = const.tile([PART, 2, PART], FP8)
            nc.vector.memset(ones[:], 1.0)
            psw = pswarm.tile([PART, 512], F32)
            for _ in range(warmup_mms):
                nc.tensor.matmul(psw[:], lhsT=warm[:, :, :PART], rhs=warm[:],
                                 start=True, stop=True, perf_mode=DR)

            blocks = [(pop, nbi) for pop in range(ppc) for nbi in range(nb)]
            xts = [xpool.tile([PART, kt, b], FP8, tag="xt", name=f"xt_{p}")
                   for p in range(ppc)]
            w1t = [wsrc.tile([PART, kh, 2, 512], FP8, tag="ws",
                             name=f"w1t_{i}") for i in range(nblocks)]
            wd4 = [wdpool.tile([PART, kh, 2, 512], FP8, tag="wd",
                               name=f"wd_{i}") for i in range(nblocks)]

            # ---- loads: weights on sync (strict deadline order), x on scalar
            def load_w(i, chunks):
                pop, nbi = blocks[i]
                c0 = 0
                for ch in chunks:
                    nc.sync.dma_start(
                        out=w1t[i][:, c0:c0 + ch, :, :],
                        in_=w1_d.ap()[pop, nbi, :, c0:c0 + ch, :, :])
                    c0 += ch
                c0 = 0
                for ch in chunks:
                    nc.sync.dma_start(
                        out=wd4[i][:, c0:c0 + ch, :, :],
                        in_=w0_d.ap()[pop, nbi, :, c0:c0 + ch, :, :])
                    c0 += ch

            # deadline order on the single weight ring: x0 m-slice 0 first
            # (block 0 is m-outer), block-0 weights in fine chunks, then the
            # rest of x0, then blocks in order with x1 before pop 1's blocks
            nc.sync.dma_start(out=xts[0][:, 0:4, :],
                              in_=xt_d.ap()[0, :, 0:4, :])
            load_w(0, [1, 1, 2, 4])
            nc.sync.dma_start(out=xts[0][:, 4:kt, :],
                              in_=xt_d.ap()[0, :, 4:kt, :])
            for i in range(1, nblocks):
                if i == nb and ppc > 1:
                    nc.sync.dma_start(out=xts[1][:], in_=xt_d.ap()[1])
                load_w(i, [kh])

            state = {}

            def prep(i):
                pop, nbi = blocks[i]
                psb = psbias.tile([PART, 512], F32, tag="psb",
                                  name=f"psb_{i}")
                if i == 0:
                    # raw bias: stream w1 through the PE (doubles as warm-up)
                    for j in range(kh):
                        nc.tensor.matmul(
                            psb[:], lhsT=ones[:], rhs=w1t[i][:, j, :, :],
                            start=(j == 0), stop=(j == kh - 1), perf_mode=DR)
                else:
                    # split bias between DVE pair-tree (tp pairs) and raw PE
                    # streaming (the rest): minimizes max(PE, DVE) per block
                    tp = tree_pairs
                    nmm = (tp // 2) + (kh - tp)
                    mmi = 0
                    if tp:
                        s8 = s8pool.tile([PART, tp, 512], FP8, tag="s8",
                                         name=f"s8_{i}")
                        nc.vector.tensor_tensor(
                            s8[:], w1t[i][:, 0:tp, 0, :], w1t[i][:, 0:tp, 1, :],
                            mybir.AluOpType.add)
                        for j in range(tp // 2):
                            nc.tensor.matmul(
                                psb[:], lhsT=ones[:],
                                rhs=s8[:, 2 * j:2 * j + 2, :],
                                start=(mmi == 0), stop=(mmi == nmm - 1),
                                perf_mode=DR)
                            mmi += 1
                    for j in range(tp, kh):
                        nc.tensor.matmul(
                            psb[:], lhsT=ones[:], rhs=w1t[i][:, j, :, :],
                            start=(mmi == 0), stop=(mmi == nmm - 1),
                            perf_mode=DR)
                        mmi += 1
                bias_sb = bpool.tile([PART, 512], F32, tag="bias",
                                     name=f"bias_{i}")
                nc.scalar.copy(bias_sb[:], psb[:])
                # wd = w0 XOR w1n (single op: saves per-op issue overhead)
                if i == 0:
                    for c in range(0, kh, 2):
                        nc.vector.tensor_tensor(
                            wd4[i][:, c:c + 2, :, :].bitcast(BIT),
                            wd4[i][:, c:c + 2, :, :].bitcast(BIT),
                            w1t[i][:, c:c + 2, :, :].bitcast(BIT),
                            mybir.AluOpType.bitwise_xor)
                else:
                    nc.vector.tensor_tensor(
                        wd4[i][:].bitcast(BIT), wd4[i][:].bitcast(BIT),
                        w1t[i][:].bitcast(BIT), mybir.AluOpType.bitwise_xor)
                state[i] = bias_sb

            def main(i):
                pop, nbi = blocks[i]
                bias_sb = state.pop(i)
                xt = xts[pop]
                wd = wd4[i]
                pss = [pspool.tile([PART, 512], F32, tag="ps",
                                   name=f"ps_{i}_{m}") for m in range(mb)]
                osl = slice(nbi * 512, (nbi + 1) * 512)

                def evac(m):
                    ot = opool.tile([PART, 512], F16, tag="ot",
                                    name=f"ot_{i}_{m}")
                    # out = ps - (-bias)
                    nc.vector.tensor_tensor(
                        ot[:], pss[m][:], bias_sb[:],
                        mybir.AluOpType.subtract)
                    msl = slice(m * PART, (m + 1) * PART)
                    if i >= late_store_from:
                        eng = nc.sync if m % 2 == 0 else nc.scalar
                    else:
                        eng = nc.gpsimd
                    eng.dma_start(out=out_d.ap()[pop, msl, osl], in_=ot[:])

                if i == nblocks - 1:
                    # m-outer: only one evac+store trails the last matmul
                    for m in range(mb):
                        msl = slice(m * PART, (m + 1) * PART)
                        for kd in range(nk):
                            nc.tensor.matmul(
                                pss[m][:], lhsT=xt[:, 2 * kd:2 * kd + 2, msl],
                                rhs=wd[:, kd, :, :], start=(kd == 0),
                                stop=(kd == nk - 1), perf_mode=DR)
                        evac(m)
                else:
                    for kd in range(nk):
                        for m in range(mb):
                            msl = slice(m * PART, (m + 1) * PART)
                            nc.tensor.matmul(
                                pss[m][:], lhsT=xt[:, 2 * kd:2 * kd + 2, msl],
                                rhs=wd[:, kd, :, :], start=(kd == 0),
                                stop=(kd == nk - 1), perf_mode=DR)
                            if kd == nk - 1:
                                evac(m)

            prep(0)
            for i in range(nblocks):
                main(i)
                if i + 1 < nblocks:
                    prep(i + 1)
    nc.compile()
    return nc


def build_nc_v2(ppc=PPC, b=B, i_dim=I, o_dim=O, n_cores=N_CORES):
    """v2: algebraic rewrite out = x@(w0-w1) + colsum(w1).

    The w1 input tensor holds -w1 (sign applied during the host fp8 cast;
    walrus rejects cce_op=subtract but accepts add):
    - wd = w0 + (-w1) computed by the gpsimd DMA inline ALU (accum_op=add)
      while loading w0 — zero compute-engine cost.
    - colsum(-w1) = -bias via an all-ones stationary matmul against the tile
      while it still holds -w1, once per o-block.
    - main pass: psum = x @ wd, half the PE work of v1; evacuated as
      psum - (-bias) with a DVE tensor_tensor subtract.
    All values stay exact: x in {0,1}, wd in {-1,0,1} (fp8 exact), bias and
    accumulation in f32 (integers < 2^24).
    """
    kt = i_dim // PART
    nb = o_dim // 512
    mb = b // PART
    DR = mybir.MatmulPerfMode.DoubleRow
    nk = kt // 2

    nc = bacc.Bacc("TRN2", target_bir_lowering=False, debug=False,
                   num_devices=n_cores)

    xt_d = nc.dram_tensor("xt", [ppc, PART, kt, b], FP8, kind="ExternalInput")
    w0_d = nc.dram_tensor("w0", [ppc, nb, PART, kt, 512], FP8, kind="ExternalInput")
    w1_d = nc.dram_tensor("w1", [ppc, nb, PART, kt, 512], FP8, kind="ExternalInput")
    out_d = nc.dram_tensor("out", [ppc, b, o_dim], F32, kind="ExternalOutput")

    with tile.TileContext(nc) as tc:
        with (
            tc.tile_pool(name="const", bufs=1) as const,
            tc.tile_pool(name="xpool", bufs=2) as xpool,
            tc.tile_pool(name="wpool", bufs=4) as wpool,
            tc.tile_pool(name="bpool", bufs=2) as bpool,
            tc.tile_pool(name="opool", bufs=4) as opool,
            tc.tile_pool(name="pspool", bufs=4, space="PSUM") as pspool,
            tc.tile_pool(name="psbias", bufs=2, space="PSUM") as psbias,
        ):
            ones = const.tile([PART, 2, PART], FP8)
            nc.vector.memset(ones[:], 1.0)
            for pop in range(ppc):
                xt = xpool.tile([PART, kt, b], FP8, tag="xt")
                nc.scalar.dma_start(out=xt[:], in_=xt_d.ap()[pop])
                for nbi in range(nb):
                    # 544-wide rows (512 data + 32 pad): keeps every SBUF write
                    # run at 512B so the accum DMA's RMW ucode accepts it (runs
                    # >512B crash the exec unit), and stops the AP optimizer
                    # from merging rows into one big run.
                    wdp = wpool.tile([PART, kt, 544], FP8, tag="w")
                    wd = wdp[:, :, :512]
                    # 1) load -w1 (sync HWDGE ring)
                    wch = min(8, kt)
                    for ch in range(0, kt, wch):
                        nc.sync.dma_start(
                            out=wd[:, ch:ch + wch, :],
                            in_=w1_d.ap()[pop, nbi, :, ch:ch + wch, :])
                    # 2) -bias = colsum(-w1) while the tile still holds -w1
                    psb = psbias.tile([PART, 512], F32)
                    for kd in range(nk):
                        ksl = slice(2 * kd, 2 * kd + 2)
                        nc.tensor.matmul(
                            psb[:], lhsT=ones[:], rhs=wd[:, ksl, :],
                            start=(kd == 0), stop=(kd == nk - 1), perf_mode=DR)
                    bias_sb = bpool.tile([PART, 512], F32, tag="bias")
                    nc.vector.tensor_copy(bias_sb[:], psb[:])
                    # 3) wd = w0 + (-w1) via DMA inline ALU (op(in,out) = in+out)
                    nc.gpsimd.dma_start(out=wd[:], in_=w0_d.ap()[pop, nbi],
                                        accum_op=mybir.AluOpType.add)
                    # 4) main pass: psum = x @ wd, evac with bias add
                    for m in range(mb):
                        ps = pspool.tile([PART, 512], F32)
                        msl = slice(m * PART, (m + 1) * PART)
                        for kd in range(nk):
                            ksl = slice(2 * kd, 2 * kd + 2)
                            nc.tensor.matmul(
                                ps[:], lhsT=xt[:, ksl, msl], rhs=wd[:, ksl, :],
                                start=(kd == 0), stop=(kd == nk - 1), perf_mode=DR)
                        ot = opool.tile([PART, 512], F32)
                        # out = psum - (-bias)
                        nc.vector.tensor_tensor(
                            ot[:], ps[:], bias_sb[:], mybir.AluOpType.subtract)
                        nc.scalar.dma_start(
                            out=out_d.ap()[pop, msl, nbi * 512:(nbi + 1) * 512],
                            in_=ot[:])
    nc.compile()
    return nc


def prep_core_inputs(x, w, core, ppc=PPC, negate_w1=False, wscale=1.0,
                     x_mmajor=False):
    """Layout-only host prep for one core: slice pops, transpose x, tile, cast.
    With negate_w1, the fp8 cast of w1 carries a sign flip (v2 sends -w1 so the
    device can form w0-w1 with the DMA ALU's accum add).  wscale selects the
    fp8 code pair used for the 0/1 booleans (v12 uses {0,+-128} so the device
    bias tree is bitwise); the device folds the 2^-7 back in during evac."""
    p0 = core * ppc
    b, i_dim = x.shape[1], x.shape[2]
    o_dim = w.shape[4]
    kt = i_dim // PART
    nb = o_dim // 512
    xs = x[p0:p0 + ppc]                       # [ppc, B, I]
    if x_mmajor:
        # [ppc, mb, 128, kt, 128]; xm[p,m,kp,kti,j] = x[p, m*128+j, kti*128+kp]
        mb = b // PART
        xt = np.ascontiguousarray(
            xs.reshape(ppc, mb, PART, kt, PART).transpose(0, 1, 4, 3, 2)
        ).astype(NP_FP8)
    else:
        # xT partition-tiled: [ppc, 128, kt, B];  xt[p, kp, kti, b] = x[p, b, kti*128+kp]
        xt = np.ascontiguousarray(
            xs.reshape(ppc, b, kt, PART).transpose(0, 3, 2, 1)
        ).astype(NP_FP8)
    ws = w[:, p0:p0 + ppc, 0]                 # [2, ppc, I, O]
    # [2, ppc, nb, 128, kt, 512]; wt[j,p,nbi,kp,kti,no] = w[j,p,kti*128+kp, nbi*512+no]
    wt = np.ascontiguousarray(
        ws.reshape(2, ppc, kt, PART, nb, 512).transpose(0, 1, 4, 3, 2, 5)
    )
    w0 = (wt[0] * wscale).astype(NP_FP8) if wscale != 1.0 else wt[0].astype(NP_FP8)
    # +0.0 normalizes -0.0 so the fp8 pattern is 0x00, not 0x80 -- the
    # XOR identity requires w1n in {+0.0, -scale} exactly.
    w1 = ((wt[1] * -wscale) + 0.0).astype(NP_FP8) if negate_w1 else wt[1].astype(NP_FP8)
    return {"xt": xt, "w0": w0, "w1": w1}


_NC_CACHE = {}

# which builder kernel() uses: 1 = concat (x@w0 + notx@w1), 2 = DMA-subtract trick
K_VERSION = int(os.environ.get("EVO_KERNEL_VERSION", "10"))
NEGATE_VERSIONS = (2, 5, 6, 7, 9, 10, 11, 12, 13)
RESHAPE_VERSIONS = (10, 11, 12, 13)
SCALE128_VERSIONS = ()
XMMAJOR_VERSIONS = ()


def _get_nc():
    if "nc" not in _NC_CACHE:
        builder = {1: build_nc, 2: build_nc_v2, 3: build_nc_v3,
                   4: build_nc_v4, 5: build_nc_v5, 6: build_nc_v6,
                   7: build_nc_v7, 9: build_nc_v9, 10: build_nc_v10,
                   11: lambda: build_nc_v10(tree_eng="gpsimd",
                                            psb_eng="scalar",
                                            late_store_from=5),
                   12: build_nc_v12,
                   13: lambda: build_nc_v12(tree_pairs=4)}[K_VERSION]
        _NC_CACHE["nc"] = builder()
    return _NC_CACHE["nc"]


def _reshape_for_v10(m):
    # v10 declares w0/w1 as [ppc, nb, 128, kh, 2, 512] (same bytes)
    for k in ("w0", "w1"):
        s = m[k].shape
        m[k] = m[k].reshape(s[0], s[1], s[2], s[3] // 2, 2, s[4])
    return m


def kernel(x, w):
    x = np.asarray(x)
    w = np.asarray(w)
    nc = _get_nc()
    wscale = 128.0 if K_VERSION in SCALE128_VERSIONS else 1.0
    in_maps = [prep_core_inputs(x, w, c,
                                negate_w1=(K_VERSION in NEGATE_VERSIONS),
                                wscale=wscale,
                                x_mmajor=(K_VERSION in XMMAJOR_VERSIONS))
               for c in range(N_CORES)]
    if K_VERSION in RESHAPE_VERSIONS:
        in_maps = [_reshape_for_v10(m) for m in in_maps]
    res = run_bass_kernel_spmd(nc, in_maps, list(range(N_CORES)))
    out = np.concatenate([res.results[c]["out"] for c in range(N_CORES)], axis=0)
    return np.ascontiguousarray(out.astype(np.float32))

